# revision 1
# baseline (speedup 1.0000x reference)
"""Trainium2 Bass kernel for single-head attention model.

Reference computation (B=4, S=2048, E=1024, fp32):
    q = query @ Wq + bq;  k = key @ Wk + bk;  v = value @ Wv + bv
    scores = (q @ k^T) / sqrt(E)
    out = softmax(scores, axis=-1) @ v

Sharding: 8 cores; core c handles batch b = c // 2, query-row half
h = c % 2 (1024 q-rows). K/V projections for the full batch are
computed redundantly on both cores of a pair (no collectives).

Device layout strategy (all matmuls run in float32r = TF32-like
single-pass mode, 1 cycle/row at free-dim >= 256):
  - host pre-transposes inputs so contraction dims land on partitions:
      xqT[e, s_q], xkT[e, s_k], xvT[e, s_k]
  - QT[e, s_q]  = Wq^T xqT   (lhsT = Wq natural layout)
  - KT[e, s_k]  = Wk^T xkT
  - V[s_k, e]   = xvT^T Wv   (lhsT = xvT slices)
  - scoresT[s_k, s_q_blk] = KT^T_slices @ QT  (transposed scores!)
  - expT = exp(scoresT / 32)  -- no max subtraction; scores are O(1)
  - out_unnorm[s_q, e] = expT^T @ V   (lhsT = expT slices, no transposes)
  - sums[s_q] via DVE partial-sum chain over s_k tiles + one tiny
    ones-matmul per 128-row group to reduce over partitions
  - out = out_unnorm * (1/sums)  (per-partition scalar), DMA out natural
"""

import sys

sys.path.insert(0, "/opt/trn_rl_repo")

from contextlib import ExitStack

import numpy as np

import concourse.bass as bass
import concourse.mybir as mybir
import concourse.tile as tile
from concourse import bacc, bass_utils

F32R = mybir.dt.float32r
F32 = mybir.dt.float32
AF = mybir.ActivationFunctionType

B, S, E = 4, 2048, 1024
N_CORES = 8
SQ = S // 2          # q rows per core
SK = S               # kv rows per core
BQ = 256             # s_q block width in phase D
NBLK = SQ // BQ      # 4 blocks
EK = E // 128        # 8 contraction tiles over e
MK = SK // 128       # 16 s_k tiles
INV_SCALE = 1.0 / float(np.sqrt(E))

_cached = {}


def _build():
    nc = bacc.Bacc("TRN2", target_bir_lowering=False, debug=False,
                   num_devices=N_CORES)

    xqT = nc.dram_tensor("xqT", [E, SQ], F32R, kind="ExternalInput").ap()
    xkT = nc.dram_tensor("xkT", [E, SK], F32R, kind="ExternalInput").ap()
    xvT = nc.dram_tensor("xvT", [E, SK], F32R, kind="ExternalInput").ap()
    wq = nc.dram_tensor("wq", [E, E], F32R, kind="ExternalInput").ap()
    wk = nc.dram_tensor("wk", [E, E], F32R, kind="ExternalInput").ap()
    wv = nc.dram_tensor("wv", [E, E], F32R, kind="ExternalInput").ap()
    # biases pre-arranged on host: b_sb[p, t] = b[t*128 + p]
    bqh = nc.dram_tensor("bqh", [128, EK], F32, kind="ExternalInput").ap()
    bkh = nc.dram_tensor("bkh", [128, EK], F32, kind="ExternalInput").ap()
    bvh = nc.dram_tensor("bvh", [1, E], F32, kind="ExternalInput").ap()
    ones_in = nc.dram_tensor("ones_in", [128, 2], F32R, kind="ExternalInput").ap()
    out = nc.dram_tensor("out", [SQ, E], F32, kind="ExternalOutput").ap()

    with tile.TileContext(nc) as tc, ExitStack() as top:
        # ---- long-lived pools (live to end of kernel) ----
        consts = top.enter_context(tc.tile_pool(name="consts", bufs=1))
        vpool = top.enter_context(tc.tile_pool(name="vpool", bufs=1))

        ones_t = consts.tile([128, 2], F32R)
        nc.sync.dma_start(ones_t[:], ones_in)
        bq_t = consts.tile([128, EK], F32)
        nc.sync.dma_start(bq_t[:], bqh)
        bk_t = consts.tile([128, EK], F32)
        nc.sync.dma_start(bk_t[:], bkh)

        # V[s_k, e]: 16 tiles of [128, E]
        v_tiles = [vpool.tile([128, E], F32R, tag=f"v{m}", name=f"v{m}") for m in range(MK)]

        # ================= phase A: V = xvT^T @ Wv + bv =================
        # xvT streamed in 512-col (s_k) blocks; V output tiles accumulate
        with tc.tile_pool(name="xvblk", bufs=3) as xvp, \
             tc.tile_pool(name="wvp", bufs=1) as wvp, \
             tc.tile_pool(name="bvp", bufs=1) as bvp, \
             tc.tile_pool(name="psA", bufs=2, space="PSUM") as psA:
            bv_row = bvp.tile([1, E], F32)
            nc.sync.dma_start(bv_row[:], bvh)
            bv_bc = bvp.tile([128, E], F32)
            nc.gpsimd.partition_broadcast(bv_bc[:], bv_row[:])
            wv_tiles = [wvp.tile([128, E], F32R, tag=f"wv{k}", name=f"wv{k}") for k in range(EK)]
            for n in range(E // 512):
                for k in range(EK):
                    nc.sync.dma_start(
                        wv_tiles[k][:, n * 512:(n + 1) * 512],
                        wv[k * 128:(k + 1) * 128, n * 512:(n + 1) * 512])
            for mb in range(SK // 512):
                xv_blk = [xvp.tile([128, 512], F32R, tag=f"xvb{k}", name=f"xvb{mb}_{k}")
                          for k in range(EK)]
                for k in range(EK):
                    nc.sync.dma_start(
                        xv_blk[k][:],
                        xvT[k * 128:(k + 1) * 128, mb * 512:(mb + 1) * 512])
                for n in range(E // 512):
                    pss = [psA.tile([128, 512], F32, tag=f"psA{i}", name=f"psA_{mb}_{n}_{i}")
                           for i in range(4)]
                    for k in range(EK):
                        for i in range(4):
                            nc.tensor.matmul(
                                pss[i][:],
                                xv_blk[k][:, i * 128:(i + 1) * 128],
                                wv_tiles[k][:, n * 512:(n + 1) * 512],
                                start=(k == 0), stop=(k == EK - 1))
                    for i in range(4):
                        m = mb * 4 + i
                        nc.vector.tensor_add(
                            v_tiles[m][:, n * 512:(n + 1) * 512],
                            pss[i][:],
                            bv_bc[:, n * 512:(n + 1) * 512])

        # ================= phase B: KT = Wk^T @ xkT + bk =================
        ktpool = top.enter_context(tc.tile_pool(name="ktpool", bufs=1))
        kt_tiles = [ktpool.tile([128, SK], F32R, tag=f"kt{m}", name=f"kt{m}")
                    for m in range(EK)]
        with tc.tile_pool(name="xkblk", bufs=2) as xkp, \
             tc.tile_pool(name="wkp", bufs=1) as wkp, \
             tc.tile_pool(name="psB", bufs=8, space="PSUM") as psB:
            wk_tiles = [wkp.tile([128, E], F32R, tag=f"wk{k}", name=f"wk{k}")
                        for k in range(EK)]
            # m-sliced DMA order: first (nb=0, m=0) group unblocks after 8 slices
            for m in range(EK):
                for k in range(EK):
                    nc.sync.dma_start(
                        wk_tiles[k][:, m * 128:(m + 1) * 128],
                        wk[k * 128:(k + 1) * 128, m * 128:(m + 1) * 128])
            for nb in range(SK // 512):
                xk_blk = [xkp.tile([128, 512], F32R, tag=f"xkb{k}", name=f"xkb{nb}_{k}")
                          for k in range(EK)]
                for k in range(EK):
                    nc.sync.dma_start(
                        xk_blk[k][:],
                        xkT[k * 128:(k + 1) * 128, nb * 512:(nb + 1) * 512])
                for m in range(EK):
                    ps = psB.tile([128, 512], F32, tag="psB")
                    for k in range(EK):
                        nc.tensor.matmul(
                            ps[:],
                            wk_tiles[k][:, m * 128:(m + 1) * 128],
                            xk_blk[k][:],
                            start=(k == 0), stop=(k == EK - 1))
                    nc.vector.tensor_scalar_add(
                        kt_tiles[m][:, nb * 512:(nb + 1) * 512],
                        ps[:], bk_t[:, m:m + 1])

        # ================= phase C: QT = Wq^T @ xqT + bq =================
        qtpool = top.enter_context(tc.tile_pool(name="qtpool", bufs=1))
        qt_tiles = [qtpool.tile([128, SQ], F32R, tag=f"qt{m}", name=f"qt{m}")
                    for m in range(EK)]
        with tc.tile_pool(name="xqp", bufs=1) as xqp, \
             tc.tile_pool(name="wqblk", bufs=2) as wqp, \
             tc.tile_pool(name="psC", bufs=8, space="PSUM") as psC:
            xq_tiles = [xqp.tile([128, SQ], F32R, tag=f"xq{k}", name=f"xq{k}") for k in range(EK)]
            for c in range(SQ // 512):
                for k in range(EK):
                    nc.sync.dma_start(
                        xq_tiles[k][:, c * 512:(c + 1) * 512],
                        xqT[k * 128:(k + 1) * 128, c * 512:(c + 1) * 512])
            for m in range(EK):
                wq_blk = [wqp.tile([128, 128], F32R, tag=f"wqb{k}", name=f"wqb{m}_{k}")
                          for k in range(EK)]
                for k in range(EK):
                    nc.sync.dma_start(
                        wq_blk[k][:],
                        wq[k * 128:(k + 1) * 128, m * 128:(m + 1) * 128])
                for n in range(SQ // 512):
                    ps = psC.tile([128, 512], F32, tag="psC")
                    for k in range(EK):
                        nc.tensor.matmul(
                            ps[:], wq_blk[k][:],
                            xq_tiles[k][:, n * 512:(n + 1) * 512],
                            start=(k == 0), stop=(k == EK - 1))
                    nc.vector.tensor_scalar_add(
                        qt_tiles[m][:, n * 512:(n + 1) * 512],
                        ps[:], bq_t[:, m:m + 1])

        # ================= phase D: attention, blocked over s_q =========
        with tc.tile_pool(name="expp", bufs=2) as expp, \
             tc.tile_pool(name="partp", bufs=2) as partp, \
             tc.tile_pool(name="outp", bufs=1) as outp, \
             tc.tile_pool(name="sumsp", bufs=2) as sumsp, \
             tc.tile_pool(name="psS", bufs=3, space="PSUM") as psS, \
             tc.tile_pool(name="psO", bufs=1, space="PSUM") as psO, \
             tc.tile_pool(name="psSum", bufs=1, space="PSUM") as psSum:
            for blk in range(NBLK):
                q0 = blk * BQ
                # scoresT[s_k, blk] = KT^T @ QT_blk ; exp -> expT tiles
                exp_tiles = []
                for m in range(MK):
                    ps = psS.tile([128, BQ], F32, tag="psS")
                    for k in range(EK):
                        nc.tensor.matmul(
                            ps[:],
                            kt_tiles[k][:, m * 128:(m + 1) * 128],
                            qt_tiles[k][:, q0:q0 + BQ],
                            start=(k == 0), stop=(k == EK - 1))
                    et = expp.tile([128, BQ], F32R, tag=f"exp{m}")
                    nc.scalar.activation(et[:], ps[:], AF.Exp, scale=INV_SCALE)
                    exp_tiles.append(et)

                # partial sums over s_k tiles (DVE chain), last write f32r
                part = partp.tile([128, BQ], F32, tag="part")
                nc.vector.tensor_add(part[:], exp_tiles[0][:].bitcast(F32),
                                     exp_tiles[1][:].bitcast(F32))
                for m in range(2, MK - 1):
                    nc.vector.tensor_add(part[:], part[:],
                                         exp_tiles[m][:].bitcast(F32))
                part_r = partp.tile([128, BQ], F32R, tag="part_r")
                nc.vector.tensor_add(part_r[:], part[:],
                                     exp_tiles[MK - 1][:].bitcast(F32))

                # out_unnorm[s_q, e] = expT^T @ V ; sums via ones-matmul
                for mi in range(BQ // 128):
                    pssum = psSum.tile([128, 2], F32, tag="pssum")
                    nc.tensor.matmul(
                        pssum[:],
                        part_r[:, mi * 128:(mi + 1) * 128],
                        ones_t[:], start=True, stop=True)
                    recip = sumsp.tile([128, 1], F32, tag="recip")
                    nc.vector.reciprocal(recip[:], pssum[:, 0:1])

                    ot = outp.tile([128, E], F32, tag=f"out{mi}")
                    for n in range(E // 512):
                        pso = psO.tile([128, 512], F32, tag=f"psO{mi}_{n}")
                        for m in range(MK):
                            nc.tensor.matmul(
                                pso[:],
                                exp_tiles[m][:, mi * 128:(mi + 1) * 128],
                                v_tiles[m][:, n * 512:(n + 1) * 512],
                                start=(m == 0), stop=(m == MK - 1))
                        nc.vector.tensor_scalar_mul(
                            ot[:, n * 512:(n + 1) * 512], pso[:], recip[:])
                    nc.sync.dma_start(
                        out[q0 + mi * 128:q0 + (mi + 1) * 128, :], ot[:])

    nc.compile()
    return nc


def _get_nc():
    if "nc" not in _cached:
        _cached["nc"] = _build()
    return _cached["nc"]


def kernel(query, key, value, Wq, bq, Wk, bk, Wv, bv, **kw):
    query = np.ascontiguousarray(np.asarray(query, dtype=np.float32))
    key = np.ascontiguousarray(np.asarray(key, dtype=np.float32))
    value = np.ascontiguousarray(np.asarray(value, dtype=np.float32))
    Wq = np.ascontiguousarray(np.asarray(Wq, dtype=np.float32))
    Wk = np.ascontiguousarray(np.asarray(Wk, dtype=np.float32))
    Wv = np.ascontiguousarray(np.asarray(Wv, dtype=np.float32))
    bq = np.asarray(bq, dtype=np.float32)
    bk = np.asarray(bk, dtype=np.float32)
    bv = np.asarray(bv, dtype=np.float32)

    bq_h = np.ascontiguousarray(bq.reshape(EK, 128).T)
    bk_h = np.ascontiguousarray(bk.reshape(EK, 128).T)
    bv_h = np.ascontiguousarray(bv.reshape(1, E))
    ones_h = np.ones((128, 2), dtype=np.float32)

    keyT = {b: np.ascontiguousarray(key[b].T) for b in range(B)}
    valT = {b: np.ascontiguousarray(value[b].T) for b in range(B)}

    in_maps = []
    for c in range(N_CORES):
        b, h = divmod(c, 2)
        qT = np.ascontiguousarray(query[b, h * SQ:(h + 1) * SQ, :].T)
        in_maps.append({
            "xqT": qT, "xkT": keyT[b], "xvT": valT[b],
            "wq": Wq, "wk": Wk, "wv": Wv,
            "bqh": bq_h, "bkh": bk_h, "bvh": bv_h,
            "ones_in": ones_h,
        })

    nc = _get_nc()
    res = bass_utils.run_bass_kernel_spmd(
        nc, in_maps, core_ids=list(range(N_CORES)), **kw)

    full = np.empty((B, S, E), dtype=np.float32)
    for c in range(N_CORES):
        b, h = divmod(c, 2)
        full[b, h * SQ:(h + 1) * SQ, :] = res.results[c]["out"]
    kernel.last_results = res
    return full



# revision 2
# speedup vs baseline: 1.3192x; 1.3192x over previous
"""Trainium2 Bass kernel for single-head attention model.

Reference computation (B=4, S=2048, E=1024, fp32):
    q = query @ Wq + bq;  k = key @ Wk + bk;  v = value @ Wv + bv
    scores = (q @ k^T) / sqrt(E)
    out = softmax(scores, axis=-1) @ v

Sharding: 8 cores; core c handles batch b = c // 2, query-row half
h = c % 2 (1024 q-rows). No collectives.

Algebraic restructure (saves ~23% of the MACs vs the direct form):
    scores_ij = x^q_i A x^k_j + g.x^k_j (+ row-const terms that cancel
    in softmax), where A = Wq Wk^T and g = Wk bq (host-computed).
    bk drops out entirely.  On the value side,
    out = softmax(scores) @ (Xv Wv + bv) = (attn @ Xv) @ Wv + bv
    since attn rows sum to 1 — Wv is applied to only the core's own
    1024 q rows instead of all 2048 kv rows.

Per-core matmul work (128x128 PE, 1 cycle/row at free>=256):
    A = WqWk^T (65536 cyc) ; q'T = A^T-contract with xqT (65536)
    scoresT (131072) ; Z^T = Xv^T-contract with exp (131072)
    O = Z Wv (65536)  => 458752 cycles ~= 191us @2.4GHz.

All matmul inputs are bf16 (host-converted); PSUM accumulates f32.
exp/Z intermediates stored bf16.  Softmax sums come from tiny
ones-matmuls accumulated over the 16 key tiles.
"""

import sys

sys.path.insert(0, "/opt/trn_rl_repo")

from contextlib import ExitStack

import ml_dtypes
import numpy as np

import concourse.bass as bass
import concourse.mybir as mybir
import concourse.tile as tile
from concourse import bacc, bass_utils

BF16 = mybir.dt.bfloat16
F32 = mybir.dt.float32
AF = mybir.ActivationFunctionType

B, S, E = 4, 2048, 1024
N_CORES = 8
SQ = S // 2          # q rows per core
BQ = 512             # s_q block width in attention phase
NBLK = SQ // BQ      # 2 blocks
EK = E // 128        # 8 tiles over e/a/c dims
MK = S // 128        # 16 s_k tiles
INV_SCALE = 1.0 / float(np.sqrt(E))

_cached = {}


def _build():
    nc = bacc.Bacc("TRN2", target_bir_lowering=False, debug=False,
                   num_devices=N_CORES)

    # host pre-transposed / pre-converted inputs (all bf16 except consts)
    wqT = nc.dram_tensor("wqT", [E, E], BF16, kind="ExternalInput").ap()
    wkT = nc.dram_tensor("wkT", [E, E], BF16, kind="ExternalInput").ap()
    xqT = nc.dram_tensor("xqT", [E, SQ], BF16, kind="ExternalInput").ap()
    xkT = nc.dram_tensor("xkT", [E, S], BF16, kind="ExternalInput").ap()
    xv = nc.dram_tensor("xv", [S, E], BF16, kind="ExternalInput").ap()
    wv = nc.dram_tensor("wv", [E, E], BF16, kind="ExternalInput").ap()
    # g = Wk @ bq arranged g_h[p, t] = g[t*128 + p]
    gh = nc.dram_tensor("gh", [128, EK], F32, kind="ExternalInput").ap()
    bvh = nc.dram_tensor("bvh", [1, E], F32, kind="ExternalInput").ap()
    ones_in = nc.dram_tensor("ones_in", [128, 2], BF16, kind="ExternalInput").ap()
    out = nc.dram_tensor("out", [SQ, E], F32, kind="ExternalOutput").ap()

    with tile.TileContext(nc) as tc, ExitStack() as top:
        # ---- long-lived pools ----
        consts = top.enter_context(tc.tile_pool(name="consts", bufs=1))
        qtpool = top.enter_context(tc.tile_pool(name="qtpool", bufs=1))
        xkpool = top.enter_context(tc.tile_pool(name="xkpool", bufs=1))
        xvpool = top.enter_context(tc.tile_pool(name="xvpool", bufs=1))
        wvpool = top.enter_context(tc.tile_pool(name="wvpool", bufs=1))

        qt_tiles = [qtpool.tile([128, SQ], BF16, tag=f"qt{m}", name=f"qt{m}")
                    for m in range(EK)]
        xk_tiles = [xkpool.tile([128, S], BF16, tag=f"xk{k}", name=f"xk{k}")
                    for k in range(EK)]
        xv_tiles = [xvpool.tile([128, E], BF16, tag=f"xv{m}", name=f"xv{m}")
                    for m in range(MK)]
        wv_tiles = [wvpool.tile([128, E], BF16, tag=f"wv{k}", name=f"wv{k}")
                    for k in range(EK)]

        with tc.tile_pool(name="wqwk", bufs=1) as wqwkp, \
             tc.tile_pool(name="apool", bufs=1) as apool, \
             tc.tile_pool(name="xqpool", bufs=1) as xqpool:
            wq_t = [wqwkp.tile([128, E], BF16, tag=f"wq{c}", name=f"wq{c}")
                    for c in range(EK)]
            wk_t = [wqwkp.tile([128, E], BF16, tag=f"wk{c}", name=f"wk{c}")
                    for c in range(EK)]
            a_tiles = [apool.tile([128, E], BF16, tag=f"a{t}", name=f"a{t}")
                       for t in range(EK)]
            xq_tiles = [xqpool.tile([128, SQ], BF16, tag=f"xq{t}", name=f"xq{t}")
                        for t in range(EK)]

            # ---- DMA issue order = consumption order ----
            for c in range(EK):
                nc.sync.dma_start(wq_t[c][:], wqT[c * 128:(c + 1) * 128, :])
                nc.sync.dma_start(wk_t[c][:], wkT[c * 128:(c + 1) * 128, :])
            g_t = consts.tile([128, EK], F32)
            nc.sync.dma_start(g_t[:], gh)
            ones_t = consts.tile([128, 2], BF16)
            nc.sync.dma_start(ones_t[:], ones_in)
            bv_row = consts.tile([1, E], F32)
            nc.sync.dma_start(bv_row[:], bvh)
            bv_bc = consts.tile([128, E], F32)
            nc.gpsimd.partition_broadcast(bv_bc[:], bv_row[:])
            for t in range(EK):
                nc.sync.dma_start(xq_tiles[t][:], xqT[t * 128:(t + 1) * 128, :])
            for k in range(EK):
                nc.sync.dma_start(xk_tiles[k][:], xkT[k * 128:(k + 1) * 128, :])
            for m in range(MK):
                nc.sync.dma_start(xv_tiles[m][:], xv[m * 128:(m + 1) * 128, :])
            for k in range(EK):
                nc.sync.dma_start(wv_tiles[k][:], wv[k * 128:(k + 1) * 128, :])

            # ====== phase A: A = Wq Wk^T  (c-outer, 8-bank PSUM wave) ======
            with tc.tile_pool(name="psA", bufs=1, space="PSUM") as psA:
                for nb in range(E // 512):
                    psa = [psA.tile([128, 512], F32, tag=f"psA{t}",
                                    name=f"psA{nb}_{t}") for t in range(EK)]
                    for c in range(EK):
                        for t in range(EK):
                            nc.tensor.matmul(
                                psa[t][:],
                                wq_t[c][:, t * 128:(t + 1) * 128],
                                wk_t[c][:, nb * 512:(nb + 1) * 512],
                                start=(c == 0), stop=(c == EK - 1))
                    for t in range(EK):
                        nc.vector.tensor_scalar_add(
                            a_tiles[t][:, nb * 512:(nb + 1) * 512],
                            psa[t][:], 0.0)

            # ====== phase Q: q''T = A^T-contraction with xqT, + g ======
            with tc.tile_pool(name="psQ", bufs=1, space="PSUM") as psQ:
                for nb in range(SQ // 512):
                    psq = [psQ.tile([128, 512], F32, tag=f"psQ{m}",
                                    name=f"psQ{nb}_{m}") for m in range(EK)]
                    for t in range(EK):
                        for m in range(EK):
                            nc.tensor.matmul(
                                psq[m][:],
                                a_tiles[t][:, m * 128:(m + 1) * 128],
                                xq_tiles[t][:, nb * 512:(nb + 1) * 512],
                                start=(t == 0), stop=(t == EK - 1))
                    for m in range(EK):
                        nc.vector.tensor_scalar_add(
                            qt_tiles[m][:, nb * 512:(nb + 1) * 512],
                            psq[m][:], g_t[:, m:m + 1])

        # ====== phase D: attention, blocked over s_q ======
        with tc.tile_pool(name="expp", bufs=1) as expp, \
             tc.tile_pool(name="ztp", bufs=2) as ztp, \
             tc.tile_pool(name="otp", bufs=1) as otp, \
             tc.tile_pool(name="rcp", bufs=2) as rcp, \
             tc.tile_pool(name="psS", bufs=2, space="PSUM") as psS, \
             tc.tile_pool(name="psZ", bufs=2, space="PSUM") as psZ, \
             tc.tile_pool(name="psO", bufs=2, space="PSUM") as psO, \
             tc.tile_pool(name="psSum", bufs=2, space="PSUM") as psSum:
            for blk in range(NBLK):
                q0 = blk * BQ
                # scoresT[s_k, blk] -> exp (bf16)
                exps = []
                for m in range(MK):
                    ps = psS.tile([128, BQ], F32, tag="psS")
                    for k in range(EK):
                        nc.tensor.matmul(
                            ps[:],
                            xk_tiles[k][:, m * 128:(m + 1) * 128],
                            qt_tiles[k][:, q0:q0 + BQ],
                            start=(k == 0), stop=(k == EK - 1))
                    et = expp.tile([128, BQ], BF16, tag=f"exp{m}",
                                   name=f"exp{blk}_{m}")
                    nc.scalar.activation(et[:], ps[:], AF.Exp, scale=INV_SCALE)
                    exps.append(et)

                # Z^T[e, i] = sum_j Xv[j, e] expT[j, i]
                zts = []
                for e_ in range(EK):
                    ps = psZ.tile([128, BQ], F32, tag="psZ")
                    for m in range(MK):
                        nc.tensor.matmul(
                            ps[:],
                            xv_tiles[m][:, e_ * 128:(e_ + 1) * 128],
                            exps[m][:],
                            start=(m == 0), stop=(m == MK - 1))
                    zt = ztp.tile([128, BQ], BF16, tag=f"zt{e_}",
                                  name=f"zt{blk}_{e_}")
                    nc.scalar.copy(zt[:], ps[:])
                    zts.append(zt)

                # softmax sums via tiny ones-matmuls; recip per i-slice
                pssum = psSum.tile([128, 2 * (BQ // 128)], F32, tag="psSum")
                recips = []
                for s in range(BQ // 128):
                    for m in range(MK):
                        nc.tensor.matmul(
                            pssum[:, 2 * s:2 * s + 2],
                            exps[m][:, s * 128:(s + 1) * 128],
                            ones_t[:],
                            start=(m == 0), stop=(m == MK - 1))
                    rc = rcp.tile([128, 1], F32, tag=f"rc{s}",
                                  name=f"rc{blk}_{s}")
                    nc.vector.reciprocal(rc[:], pssum[:, 2 * s:2 * s + 1])
                    recips.append(rc)

                # O = Z @ Wv, normalize by recip, + bv, DMA out
                for it in range(BQ // 128):
                    ot = otp.tile([128, E], F32, tag=f"ot{it}",
                                  name=f"ot{blk}_{it}")
                    for n in range(E // 512):
                        ps = psO.tile([128, 512], F32, tag="psO")
                        for e_ in range(EK):
                            nc.tensor.matmul(
                                ps[:],
                                zts[e_][:, it * 128:(it + 1) * 128],
                                wv_tiles[e_][:, n * 512:(n + 1) * 512],
                                start=(e_ == 0), stop=(e_ == EK - 1))
                        nc.scalar.activation(
                            ot[:, n * 512:(n + 1) * 512], ps[:],
                            AF.Copy, scale=recips[it][:])
                        nc.vector.tensor_add(
                            ot[:, n * 512:(n + 1) * 512],
                            ot[:, n * 512:(n + 1) * 512],
                            bv_bc[:, n * 512:(n + 1) * 512])
                    nc.sync.dma_start(
                        out[q0 + it * 128:q0 + (it + 1) * 128, :], ot[:])

    nc.compile()
    return nc


def _get_nc():
    if "nc" not in _cached:
        _cached["nc"] = _build()
    return _cached["nc"]


def _bf16(a):
    return np.ascontiguousarray(np.asarray(a, dtype=np.float32)).astype(
        ml_dtypes.bfloat16)


def kernel(query, key, value, Wq, bq, Wk, bk, Wv, bv, **kw):
    query = np.asarray(query, dtype=np.float32)
    key = np.asarray(key, dtype=np.float32)
    value = np.asarray(value, dtype=np.float32)
    Wq = np.asarray(Wq, dtype=np.float32)
    Wk = np.asarray(Wk, dtype=np.float32)
    Wv = np.asarray(Wv, dtype=np.float32)
    bq = np.asarray(bq, dtype=np.float32)
    bv = np.asarray(bv, dtype=np.float32)

    wqT_h = _bf16(Wq.T)
    wkT_h = _bf16(Wk.T)
    wv_h = _bf16(Wv)
    g = Wk @ bq                       # [E]; bk cancels in softmax
    g_h = np.ascontiguousarray(g.reshape(EK, 128).T).astype(np.float32)
    bv_h = np.ascontiguousarray(bv.reshape(1, E))
    ones_h = np.ones((128, 2), dtype=ml_dtypes.bfloat16)

    keyT = {b: _bf16(key[b].T) for b in range(B)}
    valN = {b: _bf16(value[b]) for b in range(B)}

    in_maps = []
    for c in range(N_CORES):
        b, h = divmod(c, 2)
        qT = _bf16(query[b, h * SQ:(h + 1) * SQ, :].T)
        in_maps.append({
            "wqT": wqT_h, "wkT": wkT_h, "xqT": qT,
            "xkT": keyT[b], "xv": valN[b], "wv": wv_h,
            "gh": g_h, "bvh": bv_h, "ones_in": ones_h,
        })

    nc = _get_nc()
    res = bass_utils.run_bass_kernel_spmd(
        nc, in_maps, core_ids=list(range(N_CORES)), **kw)

    full = np.empty((B, S, E), dtype=np.float32)
    for c in range(N_CORES):
        b, h = divmod(c, 2)
        full[b, h * SQ:(h + 1) * SQ, :] = res.results[c]["out"]
    kernel.last_results = res
    return full


# revision 10
# speedup vs baseline: 1.4878x; 1.1278x over previous
"""Trainium2 Bass kernel for single-head attention model.

Reference computation (B=4, S=2048, E=1024, fp32):
    q = query @ Wq + bq;  k = key @ Wk + bk;  v = value @ Wv + bv
    scores = (q @ k^T) / sqrt(E)
    out = softmax(scores, axis=-1) @ v

Sharding: 8 cores; core c handles batch b = c // 2, query-row half
h = c % 2 (1024 q-rows). No collectives.

Algebraic restructure (saves ~23% of the MACs vs the direct form):
    scores_ij = x^q_i A x^k_j + g.x^k_j (+ row-const terms that cancel
    in softmax), where A = Wq Wk^T and g = Wk bq (host-computed).
    bk drops out entirely.  On the value side,
    out = softmax(scores) @ (Xv Wv + bv) = (attn @ Xv) @ Wv + bv
    since attn rows sum to 1 — Wv is applied to only the core's own
    1024 q rows instead of all 2048 kv rows.

Per-core matmul work (128x128 PE, 1 cycle/row at free>=256):
    A = WqWk^T (65536 cyc) ; q'T = A^T-contract with xqT (65536)
    scoresT (131072) ; Z^T = Xv^T-contract with exp (131072)
    O = Z Wv (65536)  => 458752 cycles ~= 191us @2.4GHz.

All matmul inputs are bf16 (host-converted); PSUM accumulates f32.
exp/Z intermediates stored bf16.  Softmax sums come from tiny
ones-matmuls accumulated over the 16 key tiles.
"""

import sys

sys.path.insert(0, "/opt/trn_rl_repo")

from contextlib import ExitStack

import ml_dtypes
import numpy as np

import concourse.bass as bass
import concourse.mybir as mybir
import concourse.tile as tile
from concourse import bacc, bass_utils

BF16 = mybir.dt.bfloat16
F32 = mybir.dt.float32
AF = mybir.ActivationFunctionType

B, S, E = 4, 2048, 1024
N_CORES = 8
SQ = S // 2          # q rows per core
BQ = 512             # s_q block width in attention phase
NBLK = SQ // BQ      # 2 blocks
EK = E // 128        # 8 tiles over e/a/c dims
MK = S // 128        # 16 s_k tiles
INV_SCALE = 1.0 / float(np.sqrt(E))

_cached = {}


def _build():
    nc = bacc.Bacc("TRN2", target_bir_lowering=False, debug=False,
                   num_devices=N_CORES)

    # host pre-transposed / pre-converted inputs (all bf16 except consts)
    wqT = nc.dram_tensor("wqT", [E, E], BF16, kind="ExternalInput").ap()
    wkT = nc.dram_tensor("wkT", [E, E], BF16, kind="ExternalInput").ap()
    xqT = nc.dram_tensor("xqT", [E, SQ], BF16, kind="ExternalInput").ap()
    xkT = nc.dram_tensor("xkT", [E, S], BF16, kind="ExternalInput").ap()
    xv = nc.dram_tensor("xv", [S, E], BF16, kind="ExternalInput").ap()
    wv = nc.dram_tensor("wv", [E, E], BF16, kind="ExternalInput").ap()
    # g = Wk @ bq arranged g_h[p, t] = g[t*128 + p]
    gh = nc.dram_tensor("gh", [128, EK], F32, kind="ExternalInput").ap()
    bvh = nc.dram_tensor("bvh", [1, E], F32, kind="ExternalInput").ap()
    out = nc.dram_tensor("out", [SQ, E], F32, kind="ExternalOutput").ap()

    with tile.TileContext(nc) as tc, ExitStack() as top:
        # ---- long-lived pools ----
        consts = top.enter_context(tc.tile_pool(name="consts", bufs=1))
        qtpool = top.enter_context(tc.tile_pool(name="qtpool", bufs=1))
        xkpool = top.enter_context(tc.tile_pool(name="xkpool", bufs=1))
        xvpool = top.enter_context(tc.tile_pool(name="xvpool", bufs=1))
        wvpool = top.enter_context(tc.tile_pool(name="wvpool", bufs=1))

        qt_tiles = [qtpool.tile([128, SQ], BF16, tag=f"qt{m}", name=f"qt{m}")
                    for m in range(EK)]
        xk_tiles = [xkpool.tile([128, S], BF16, tag=f"xk{k}", name=f"xk{k}")
                    for k in range(EK)]
        xv_tiles = [xvpool.tile([128, E], BF16, tag=f"xv{m}", name=f"xv{m}")
                    for m in range(MK)]
        wv_tiles = [wvpool.tile([128, E], BF16, tag=f"wv{k}", name=f"wv{k}")
                    for k in range(EK)]

        with tc.tile_pool(name="wqwk", bufs=1) as wqwkp, \
             tc.tile_pool(name="apool", bufs=1) as apool, \
             tc.tile_pool(name="xqpool", bufs=1) as xqpool:
            wq_t = [wqwkp.tile([128, E], BF16, tag=f"wq{c}", name=f"wq{c}")
                    for c in range(EK)]
            wk_t = [wqwkp.tile([128, E], BF16, tag=f"wk{c}", name=f"wk{c}")
                    for c in range(EK)]
            a_tiles = [apool.tile([128, E], BF16, tag=f"a{t}", name=f"a{t}")
                       for t in range(EK)]
            xq_tiles = [xqpool.tile([128, SQ], BF16, tag=f"xq{t}", name=f"xq{t}")
                        for t in range(EK)]

            # ---- DMA issue order = consumption order ----
            for c in range(EK):
                nc.sync.dma_start(wq_t[c][:], wqT[c * 128:(c + 1) * 128, :])
                nc.sync.dma_start(wk_t[c][:], wkT[c * 128:(c + 1) * 128, :])
            g_t = consts.tile([128, EK], F32)
            nc.sync.dma_start(g_t[:], gh)
            ones_r = consts.tile([128, 256], F32)
            nc.vector.memset(ones_r[:], 1.0)
            bv_row = consts.tile([1, E], F32)
            nc.sync.dma_start(bv_row[:], bvh)
            bv_bc = consts.tile([128, E], F32)
            nc.gpsimd.partition_broadcast(bv_bc[:], bv_row[:])
            for t in range(EK):
                nc.sync.dma_start(xq_tiles[t][:], xqT[t * 128:(t + 1) * 128, :])
            for k in range(EK):
                nc.sync.dma_start(xk_tiles[k][:], xkT[k * 128:(k + 1) * 128, :])
            for m in range(MK):
                nc.sync.dma_start(xv_tiles[m][:], xv[m * 128:(m + 1) * 128, :])
            for k in range(EK):
                nc.sync.dma_start(wv_tiles[k][:], wv[k * 128:(k + 1) * 128, :])

            # ====== phase A: A = Wq Wk^T  (c-outer, 8-bank PSUM wave) ======
            with tc.tile_pool(name="psA", bufs=1, space="PSUM") as psA:
                for nb in range(E // 512):
                    psa = [psA.tile([128, 512], F32, tag=f"psA{t}",
                                    name=f"psA{nb}_{t}") for t in range(EK)]
                    for c in range(EK):
                        for t in range(EK):
                            nc.tensor.matmul(
                                psa[t][:],
                                wq_t[c][:, t * 128:(t + 1) * 128],
                                wk_t[c][:, nb * 512:(nb + 1) * 512],
                                start=(c == 0), stop=(c == EK - 1))
                    # drain copies split across DVE/Act to shorten the tail
                    for t in range(EK):
                        dst = a_tiles[t][:, nb * 512:(nb + 1) * 512]
                        if t % 2 == 0:
                            nc.vector.tensor_scalar_add(dst, psa[t][:], 0.0)
                        else:
                            nc.scalar.copy(dst, psa[t][:])

            # ====== phase Q: q''T = A^T-contraction with xqT, + g ======
            with tc.tile_pool(name="psQ", bufs=1, space="PSUM") as psQ:
                for nb in range(SQ // 512):
                    psq = [psQ.tile([128, 512], F32, tag=f"psQ{m}",
                                    name=f"psQ{nb}_{m}") for m in range(EK)]
                    for t in range(EK):
                        for m in range(EK):
                            nc.tensor.matmul(
                                psq[m][:],
                                a_tiles[t][:, m * 128:(m + 1) * 128],
                                xq_tiles[t][:, nb * 512:(nb + 1) * 512],
                                start=(t == 0), stop=(t == EK - 1))
                    for m in range(EK):
                        dst = qt_tiles[m][:, nb * 512:(nb + 1) * 512]
                        if m % 2 == 0:
                            nc.vector.tensor_scalar_add(
                                dst, psq[m][:], g_t[:, m:m + 1])
                        else:
                            nc.scalar.activation(
                                dst, psq[m][:], AF.Identity,
                                bias=g_t[:, m:m + 1])

        # ====== phase D: attention, blocked over s_q ======
        with tc.tile_pool(name="expp", bufs=1) as expp, \
             tc.tile_pool(name="ztp", bufs=2) as ztp, \
             tc.tile_pool(name="otp", bufs=1) as otp, \
             tc.tile_pool(name="partp", bufs=2) as partp, \
             tc.tile_pool(name="rcp", bufs=2) as rcp, \
             tc.tile_pool(name="psS", bufs=2, space="PSUM") as psS, \
             tc.tile_pool(name="psZ", bufs=2, space="PSUM") as psZ, \
             tc.tile_pool(name="psO", bufs=2, space="PSUM") as psO, \
             tc.tile_pool(name="psSum", bufs=1, space="PSUM") as psSum:
            for blk in range(NBLK):
                q0 = blk * BQ
                # scoresT[s_k, blk] -> exp (bf16)
                exps = []
                for m in range(MK):
                    ps = psS.tile([128, BQ], F32, tag="psS")
                    for k in range(EK):
                        nc.tensor.matmul(
                            ps[:],
                            xk_tiles[k][:, m * 128:(m + 1) * 128],
                            qt_tiles[k][:, q0:q0 + BQ],
                            start=(k == 0), stop=(k == EK - 1))
                    et = expp.tile([128, BQ], BF16, tag=f"exp{m}",
                                   name=f"exp{blk}_{m}")
                    nc.scalar.activation(et[:], ps[:], AF.Exp, scale=INV_SCALE)
                    exps.append(et)

                # partial sums over s_k tiles (DVE chain), overlaps ZT below
                part = partp.tile([128, BQ], F32, tag="part",
                                  name=f"part{blk}")
                nc.vector.tensor_add(part[:], exps[0][:], exps[1][:])
                for m in range(2, MK - 1):
                    nc.vector.tensor_add(part[:], part[:], exps[m][:])
                part_r = partp.tile([128, BQ], mybir.dt.float32r, tag="part_r",
                                    name=f"part_r{blk}")
                nc.vector.tensor_add(part_r[:], part[:], exps[MK - 1][:])

                # Z^T[e, i] = sum_j Xv[j, e] expT[j, i]
                zts = []
                for e_ in range(EK):
                    ps = psZ.tile([128, BQ], F32, tag="psZ")
                    for m in range(MK):
                        nc.tensor.matmul(
                            ps[:],
                            xv_tiles[m][:, e_ * 128:(e_ + 1) * 128],
                            exps[m][:],
                            start=(m == 0), stop=(m == MK - 1))
                    zt = ztp.tile([128, BQ], BF16, tag=f"zt{e_}",
                                  name=f"zt{blk}_{e_}")
                    nc.scalar.copy(zt[:], ps[:])
                    zts.append(zt)

                # partition-reduce of part_r via 256-wide ones-matmuls
                # (real-size matmuls keep the PE p-state stretch alive)
                pssum = psSum.tile([128, 256 * (BQ // 128)], F32, tag="psSum")
                recips = []
                for s in range(BQ // 128):
                    nc.tensor.matmul(
                        pssum[:, s * 256:(s + 1) * 256],
                        part_r[:, s * 128:(s + 1) * 128],
                        ones_r[:].bitcast(mybir.dt.float32r),
                        start=True, stop=True)
                    rc = rcp.tile([128, 1], F32, tag=f"rc{s}",
                                  name=f"rc{blk}_{s}")
                    nc.vector.reciprocal(rc[:], pssum[:, s * 256:s * 256 + 1])
                    recips.append(rc)

                # O = Z @ Wv, normalize by recip, + bv, DMA out
                for it in range(BQ // 128):
                    ot = otp.tile([128, E], F32, tag=f"ot{it}",
                                  name=f"ot{blk}_{it}")
                    for n in range(E // 512):
                        ps = psO.tile([128, 512], F32, tag="psO")
                        for e_ in range(EK):
                            nc.tensor.matmul(
                                ps[:],
                                zts[e_][:, it * 128:(it + 1) * 128],
                                wv_tiles[e_][:, n * 512:(n + 1) * 512],
                                start=(e_ == 0), stop=(e_ == EK - 1))
                        nc.scalar.activation(
                            ot[:, n * 512:(n + 1) * 512], ps[:],
                            AF.Copy, scale=recips[it][:])
                        nc.vector.tensor_add(
                            ot[:, n * 512:(n + 1) * 512],
                            ot[:, n * 512:(n + 1) * 512],
                            bv_bc[:, n * 512:(n + 1) * 512])
                    nc.sync.dma_start(
                        out[q0 + it * 128:q0 + (it + 1) * 128, :], ot[:])

    nc.compile()
    return nc


def _get_nc():
    if "nc" not in _cached:
        _cached["nc"] = _build()
    return _cached["nc"]


def _bf16(a):
    return np.ascontiguousarray(np.asarray(a, dtype=np.float32)).astype(
        ml_dtypes.bfloat16)


def kernel(query, key, value, Wq, bq, Wk, bk, Wv, bv, **kw):
    query = np.asarray(query, dtype=np.float32)
    key = np.asarray(key, dtype=np.float32)
    value = np.asarray(value, dtype=np.float32)
    Wq = np.asarray(Wq, dtype=np.float32)
    Wk = np.asarray(Wk, dtype=np.float32)
    Wv = np.asarray(Wv, dtype=np.float32)
    bq = np.asarray(bq, dtype=np.float32)
    bv = np.asarray(bv, dtype=np.float32)

    wqT_h = _bf16(Wq.T)
    wkT_h = _bf16(Wk.T)
    wv_h = _bf16(Wv)
    g = Wk @ bq                       # [E]; bk cancels in softmax
    g_h = np.ascontiguousarray(g.reshape(EK, 128).T).astype(np.float32)
    bv_h = np.ascontiguousarray(bv.reshape(1, E))

    keyT = {b: _bf16(key[b].T) for b in range(B)}
    valN = {b: _bf16(value[b]) for b in range(B)}

    in_maps = []
    for c in range(N_CORES):
        b, h = divmod(c, 2)
        qT = _bf16(query[b, h * SQ:(h + 1) * SQ, :].T)
        in_maps.append({
            "wqT": wqT_h, "wkT": wkT_h, "xqT": qT,
            "xkT": keyT[b], "xv": valN[b], "wv": wv_h,
            "gh": g_h, "bvh": bv_h,
        })

    nc = _get_nc()
    res = bass_utils.run_bass_kernel_spmd(
        nc, in_maps, core_ids=list(range(N_CORES)), **kw)

    full = np.empty((B, S, E), dtype=np.float32)
    for c in range(N_CORES):
        b, h = divmod(c, 2)
        full[b, h * SQ:(h + 1) * SQ, :] = res.results[c]["out"]
    kernel.last_results = res
    return full


# revision 12
# speedup vs baseline: 1.4884x; 1.0004x over previous
"""Trainium2 Bass kernel for single-head attention model.

Reference computation (B=4, S=2048, E=1024, fp32):
    q = query @ Wq + bq;  k = key @ Wk + bk;  v = value @ Wv + bv
    scores = (q @ k^T) / sqrt(E)
    out = softmax(scores, axis=-1) @ v

Sharding: 8 cores; core c handles batch b = c // 2, query-row half
h = c % 2 (1024 q-rows). No collectives.

Algebraic restructure (saves ~23% of the MACs vs the direct form):
    scores_ij = x^q_i A x^k_j + g.x^k_j (+ row-const terms that cancel
    in softmax), where A = Wq Wk^T and g = Wk bq (host-computed).
    bk drops out entirely.  On the value side,
    out = softmax(scores) @ (Xv Wv + bv) = (attn @ Xv) @ Wv + bv
    since attn rows sum to 1 — Wv is applied to only the core's own
    1024 q rows instead of all 2048 kv rows.

Per-core matmul work (128x128 PE, 1 cycle/row at free>=256):
    A = WqWk^T (65536 cyc) ; q'T = A^T-contract with xqT (65536)
    scoresT (131072) ; Z^T = Xv^T-contract with exp (131072)
    O = Z Wv (65536)  => 458752 cycles ~= 191us @2.4GHz.

All matmul inputs are bf16 (host-converted); PSUM accumulates f32.
exp/Z intermediates stored bf16.  Softmax sums come from tiny
ones-matmuls accumulated over the 16 key tiles.
"""

import sys

sys.path.insert(0, "/opt/trn_rl_repo")

from contextlib import ExitStack

import ml_dtypes
import numpy as np

import concourse.bass as bass
import concourse.mybir as mybir
import concourse.tile as tile
from concourse import bacc, bass_utils

BF16 = mybir.dt.bfloat16
F32 = mybir.dt.float32
AF = mybir.ActivationFunctionType

B, S, E = 4, 2048, 1024
N_CORES = 8
SQ = S // 2          # q rows per core
BQ = 512             # s_q block width in attention phase
NBLK = SQ // BQ      # 2 blocks
EK = E // 128        # 8 tiles over e/a/c dims
MK = S // 128        # 16 s_k tiles
INV_SCALE = 1.0 / float(np.sqrt(E))

_cached = {}


def _build():
    nc = bacc.Bacc("TRN2", target_bir_lowering=False, debug=False,
                   num_devices=N_CORES)

    # host pre-transposed / pre-converted inputs (all bf16 except consts)
    wqT = nc.dram_tensor("wqT", [E, E], BF16, kind="ExternalInput").ap()
    wkT = nc.dram_tensor("wkT", [E, E], BF16, kind="ExternalInput").ap()
    xqT = nc.dram_tensor("xqT", [E, SQ], BF16, kind="ExternalInput").ap()
    xkT = nc.dram_tensor("xkT", [E, S], BF16, kind="ExternalInput").ap()
    xv = nc.dram_tensor("xv", [S, E], BF16, kind="ExternalInput").ap()
    wv = nc.dram_tensor("wv", [E, E], BF16, kind="ExternalInput").ap()
    # g = Wk @ bq arranged g_h[p, t] = g[t*128 + p]
    gh = nc.dram_tensor("gh", [128, EK], F32, kind="ExternalInput").ap()
    bvh = nc.dram_tensor("bvh", [1, E], F32, kind="ExternalInput").ap()
    out = nc.dram_tensor("out", [SQ, E], F32, kind="ExternalOutput").ap()

    with tile.TileContext(nc) as tc, ExitStack() as top:
        # ---- long-lived pools ----
        consts = top.enter_context(tc.tile_pool(name="consts", bufs=1))
        qtpool = top.enter_context(tc.tile_pool(name="qtpool", bufs=1))
        xkpool = top.enter_context(tc.tile_pool(name="xkpool", bufs=1))
        xvpool = top.enter_context(tc.tile_pool(name="xvpool", bufs=1))
        wvpool = top.enter_context(tc.tile_pool(name="wvpool", bufs=1))

        qt_tiles = [qtpool.tile([128, SQ], BF16, tag=f"qt{m}", name=f"qt{m}")
                    for m in range(EK)]
        xk_tiles = [xkpool.tile([128, S], BF16, tag=f"xk{k}", name=f"xk{k}")
                    for k in range(EK)]
        xv_tiles = [xvpool.tile([128, E], BF16, tag=f"xv{m}", name=f"xv{m}")
                    for m in range(MK)]
        wv_tiles = [wvpool.tile([128, E], BF16, tag=f"wv{k}", name=f"wv{k}")
                    for k in range(EK)]

        with tc.tile_pool(name="wqwk", bufs=1) as wqwkp, \
             tc.tile_pool(name="apool", bufs=1) as apool, \
             tc.tile_pool(name="xqpool", bufs=1) as xqpool:
            wq_t = [wqwkp.tile([128, E], BF16, tag=f"wq{c}", name=f"wq{c}")
                    for c in range(EK)]
            wk_t = [wqwkp.tile([128, E], BF16, tag=f"wk{c}", name=f"wk{c}")
                    for c in range(EK)]
            a_tiles = [apool.tile([128, E], BF16, tag=f"a{t}", name=f"a{t}")
                       for t in range(EK)]
            xq_tiles = [xqpool.tile([128, SQ], BF16, tag=f"xq{t}", name=f"xq{t}")
                        for t in range(EK)]

            # ---- DMA issue order = consumption order ----
            for c in range(EK):
                nc.sync.dma_start(wq_t[c][:], wqT[c * 128:(c + 1) * 128, :])
                nc.sync.dma_start(wk_t[c][:], wkT[c * 128:(c + 1) * 128, :])
            g_t = consts.tile([128, EK], F32)
            nc.sync.dma_start(g_t[:], gh)
            ones_r = consts.tile([128, 256], F32)
            nc.vector.memset(ones_r[:], 1.0)
            bv_row = consts.tile([1, E], F32)
            nc.sync.dma_start(bv_row[:], bvh)
            bv_bc = consts.tile([128, E], F32)
            nc.gpsimd.partition_broadcast(bv_bc[:], bv_row[:])
            for t in range(EK):
                nc.sync.dma_start(xq_tiles[t][:], xqT[t * 128:(t + 1) * 128, :])
            for k in range(EK):
                nc.sync.dma_start(xk_tiles[k][:], xkT[k * 128:(k + 1) * 128, :])
            for m in range(MK):
                nc.sync.dma_start(xv_tiles[m][:], xv[m * 128:(m + 1) * 128, :])
            for k in range(EK):
                nc.sync.dma_start(wv_tiles[k][:], wv[k * 128:(k + 1) * 128, :])

            # ====== phase A: A = Wq Wk^T  (c-outer, 4-bank half-waves) ======
            with tc.tile_pool(name="psA", bufs=1, space="PSUM") as psA:
                for nb in range(E // 512):
                    for th in range(2):
                        ts_ = range(th * 4, th * 4 + 4)
                        psa = {t: psA.tile([128, 512], F32, tag=f"psA{t % 4}",
                                           name=f"psA{nb}_{t}") for t in ts_}
                        for c in range(EK):
                            for t in ts_:
                                nc.tensor.matmul(
                                    psa[t][:],
                                    wq_t[c][:, t * 128:(t + 1) * 128],
                                    wk_t[c][:, nb * 512:(nb + 1) * 512],
                                    start=(c == 0), stop=(c == EK - 1))
                        # drain copies split across DVE/Act
                        for t in ts_:
                            dst = a_tiles[t][:, nb * 512:(nb + 1) * 512]
                            if t % 2 == 0:
                                nc.vector.tensor_scalar_add(dst, psa[t][:], 0.0)
                            else:
                                nc.scalar.copy(dst, psa[t][:])

            # ====== phase Q: q''T = A^T-contraction with xqT, + g ======
            with tc.tile_pool(name="psQ", bufs=1, space="PSUM") as psQ:
                for nb in range(SQ // 512):
                    for mh in range(2):
                        ms_ = range(mh * 4, mh * 4 + 4)
                        psq = {m: psQ.tile([128, 512], F32, tag=f"psQ{m % 4}",
                                           name=f"psQ{nb}_{m}") for m in ms_}
                        for t in range(EK):
                            for m in ms_:
                                nc.tensor.matmul(
                                    psq[m][:],
                                    a_tiles[t][:, m * 128:(m + 1) * 128],
                                    xq_tiles[t][:, nb * 512:(nb + 1) * 512],
                                    start=(t == 0), stop=(t == EK - 1))
                        for m in ms_:
                            dst = qt_tiles[m][:, nb * 512:(nb + 1) * 512]
                            if m % 2 == 0:
                                nc.vector.tensor_scalar_add(
                                    dst, psq[m][:], g_t[:, m:m + 1])
                            else:
                                nc.scalar.activation(
                                    dst, psq[m][:], AF.Identity,
                                    bias=g_t[:, m:m + 1])

        # ====== phase D: attention, blocked over s_q ======
        with tc.tile_pool(name="expp", bufs=1) as expp, \
             tc.tile_pool(name="ztp", bufs=2) as ztp, \
             tc.tile_pool(name="otp", bufs=1) as otp, \
             tc.tile_pool(name="partp", bufs=2) as partp, \
             tc.tile_pool(name="rcp", bufs=2) as rcp, \
             tc.tile_pool(name="psS", bufs=2, space="PSUM") as psS, \
             tc.tile_pool(name="psZ", bufs=2, space="PSUM") as psZ, \
             tc.tile_pool(name="psO", bufs=2, space="PSUM") as psO, \
             tc.tile_pool(name="psSum", bufs=1, space="PSUM") as psSum:
            for blk in range(NBLK):
                q0 = blk * BQ
                # scoresT[s_k, blk] -> exp (bf16)
                exps = []
                for m in range(MK):
                    ps = psS.tile([128, BQ], F32, tag="psS")
                    for k in range(EK):
                        nc.tensor.matmul(
                            ps[:],
                            xk_tiles[k][:, m * 128:(m + 1) * 128],
                            qt_tiles[k][:, q0:q0 + BQ],
                            start=(k == 0), stop=(k == EK - 1))
                    et = expp.tile([128, BQ], BF16, tag=f"exp{m}",
                                   name=f"exp{blk}_{m}")
                    nc.scalar.activation(et[:], ps[:], AF.Exp, scale=INV_SCALE)
                    exps.append(et)

                # partial sums over s_k tiles (DVE chain), overlaps ZT below
                part = partp.tile([128, BQ], F32, tag="part",
                                  name=f"part{blk}")
                nc.vector.tensor_add(part[:], exps[0][:], exps[1][:])
                for m in range(2, MK - 1):
                    nc.vector.tensor_add(part[:], part[:], exps[m][:])
                part_r = partp.tile([128, BQ], mybir.dt.float32r, tag="part_r",
                                    name=f"part_r{blk}")
                nc.vector.tensor_add(part_r[:], part[:], exps[MK - 1][:])

                # Z^T[e, i] = sum_j Xv[j, e] expT[j, i]
                zts = []
                for e_ in range(EK):
                    ps = psZ.tile([128, BQ], F32, tag="psZ")
                    for m in range(MK):
                        nc.tensor.matmul(
                            ps[:],
                            xv_tiles[m][:, e_ * 128:(e_ + 1) * 128],
                            exps[m][:],
                            start=(m == 0), stop=(m == MK - 1))
                    zt = ztp.tile([128, BQ], BF16, tag=f"zt{e_}",
                                  name=f"zt{blk}_{e_}")
                    nc.scalar.copy(zt[:], ps[:])
                    zts.append(zt)

                # partition-reduce of part_r via 256-wide ones-matmuls
                # (real-size matmuls keep the PE p-state stretch alive)
                pssum = psSum.tile([128, 256 * (BQ // 128)], F32, tag="psSum")
                recips = []
                for s in range(BQ // 128):
                    nc.tensor.matmul(
                        pssum[:, s * 256:(s + 1) * 256],
                        part_r[:, s * 128:(s + 1) * 128],
                        ones_r[:].bitcast(mybir.dt.float32r),
                        start=True, stop=True)
                    rc = rcp.tile([128, 1], F32, tag=f"rc{s}",
                                  name=f"rc{blk}_{s}")
                    nc.vector.reciprocal(rc[:], pssum[:, s * 256:s * 256 + 1])
                    recips.append(rc)

                # O = Z @ Wv, normalize by recip, + bv, DMA out
                for it in range(BQ // 128):
                    ot = otp.tile([128, E], F32, tag=f"ot{it}",
                                  name=f"ot{blk}_{it}")
                    for n in range(E // 512):
                        ps = psO.tile([128, 512], F32, tag="psO")
                        for e_ in range(EK):
                            nc.tensor.matmul(
                                ps[:],
                                zts[e_][:, it * 128:(it + 1) * 128],
                                wv_tiles[e_][:, n * 512:(n + 1) * 512],
                                start=(e_ == 0), stop=(e_ == EK - 1))
                        nc.scalar.activation(
                            ot[:, n * 512:(n + 1) * 512], ps[:],
                            AF.Copy, scale=recips[it][:])
                        nc.vector.tensor_add(
                            ot[:, n * 512:(n + 1) * 512],
                            ot[:, n * 512:(n + 1) * 512],
                            bv_bc[:, n * 512:(n + 1) * 512])
                        # per-half DMA keeps the end-of-kernel tail short
                        nc.sync.dma_start(
                            out[q0 + it * 128:q0 + (it + 1) * 128,
                                n * 512:(n + 1) * 512],
                            ot[:, n * 512:(n + 1) * 512])

    nc.compile()
    return nc


def _get_nc():
    if "nc" not in _cached:
        _cached["nc"] = _build()
    return _cached["nc"]


def _bf16(a):
    return np.ascontiguousarray(np.asarray(a, dtype=np.float32)).astype(
        ml_dtypes.bfloat16)


def kernel(query, key, value, Wq, bq, Wk, bk, Wv, bv, **kw):
    query = np.asarray(query, dtype=np.float32)
    key = np.asarray(key, dtype=np.float32)
    value = np.asarray(value, dtype=np.float32)
    Wq = np.asarray(Wq, dtype=np.float32)
    Wk = np.asarray(Wk, dtype=np.float32)
    Wv = np.asarray(Wv, dtype=np.float32)
    bq = np.asarray(bq, dtype=np.float32)
    bv = np.asarray(bv, dtype=np.float32)

    wqT_h = _bf16(Wq.T)
    wkT_h = _bf16(Wk.T)
    wv_h = _bf16(Wv)
    g = Wk @ bq                       # [E]; bk cancels in softmax
    g_h = np.ascontiguousarray(g.reshape(EK, 128).T).astype(np.float32)
    bv_h = np.ascontiguousarray(bv.reshape(1, E))

    keyT = {b: _bf16(key[b].T) for b in range(B)}
    valN = {b: _bf16(value[b]) for b in range(B)}

    in_maps = []
    for c in range(N_CORES):
        b, h = divmod(c, 2)
        qT = _bf16(query[b, h * SQ:(h + 1) * SQ, :].T)
        in_maps.append({
            "wqT": wqT_h, "wkT": wkT_h, "xqT": qT,
            "xkT": keyT[b], "xv": valN[b], "wv": wv_h,
            "gh": g_h, "bvh": bv_h,
        })

    nc = _get_nc()
    res = bass_utils.run_bass_kernel_spmd(
        nc, in_maps, core_ids=list(range(N_CORES)), **kw)

    full = np.empty((B, S, E), dtype=np.float32)
    for c in range(N_CORES):
        b, h = divmod(c, 2)
        full[b, h * SQ:(h + 1) * SQ, :] = res.results[c]["out"]
    kernel.last_results = res
    return full


# revision 13
# speedup vs baseline: 1.5285x; 1.0269x over previous
"""Trainium2 Bass kernel for single-head attention model.

Reference computation (B=4, S=2048, E=1024, fp32):
    q = query @ Wq + bq;  k = key @ Wk + bk;  v = value @ Wv + bv
    scores = (q @ k^T) / sqrt(E)
    out = softmax(scores, axis=-1) @ v

Sharding: 8 cores; core c handles batch b = c // 2, query-row half
h = c % 2 (1024 q-rows). No collectives.

Algebraic restructure (saves ~23% of the MACs vs the direct form):
    scores_ij = x^q_i A x^k_j + g.x^k_j (+ row-const terms that cancel
    in softmax), where A = Wq Wk^T and g = Wk bq (host-computed).
    bk drops out entirely.  On the value side,
    out = softmax(scores) @ (Xv Wv + bv) = (attn @ Xv) @ Wv + bv
    since attn rows sum to 1 — Wv is applied to only the core's own
    1024 q rows instead of all 2048 kv rows.

Per-core matmul work (128x128 PE, 1 cycle/row at free>=256):
    A = WqWk^T (65536 cyc) ; q'T = A^T-contract with xqT (65536)
    scoresT (131072) ; Z^T = Xv^T-contract with exp (131072)
    O = Z Wv (65536)  => 458752 cycles ~= 191us @2.4GHz.

All matmul inputs are bf16 (host-converted); PSUM accumulates f32.
exp/Z intermediates stored bf16.  Softmax sums come from tiny
ones-matmuls accumulated over the 16 key tiles.
"""

import sys

sys.path.insert(0, "/opt/trn_rl_repo")

from contextlib import ExitStack

import ml_dtypes
import numpy as np

import concourse.bass as bass
import concourse.mybir as mybir
import concourse.tile as tile
from concourse import bacc, bass_utils

BF16 = mybir.dt.bfloat16
F32 = mybir.dt.float32
AF = mybir.ActivationFunctionType

B, S, E = 4, 2048, 1024
N_CORES = 8
SQ = S // 2          # q rows per core
BQ = 512             # s_q block width in attention phase
NBLK = SQ // BQ      # 2 blocks
EK = E // 128        # 8 tiles over e/a/c dims
MK = S // 128        # 16 s_k tiles
INV_SCALE = 1.0 / float(np.sqrt(E))

_cached = {}


def _build():
    nc = bacc.Bacc("TRN2", target_bir_lowering=False, debug=False,
                   num_devices=N_CORES)

    # host pre-transposed / pre-converted inputs (all bf16 except consts)
    wqT = nc.dram_tensor("wqT", [E, E], BF16, kind="ExternalInput").ap()
    wkT = nc.dram_tensor("wkT", [E, E], BF16, kind="ExternalInput").ap()
    xqT = nc.dram_tensor("xqT", [E, SQ], BF16, kind="ExternalInput").ap()
    xkT = nc.dram_tensor("xkT", [E, S], BF16, kind="ExternalInput").ap()
    xv = nc.dram_tensor("xv", [S, E], BF16, kind="ExternalInput").ap()
    wv = nc.dram_tensor("wv", [E, E], BF16, kind="ExternalInput").ap()
    # g = Wk @ bq arranged g_h[p, t] = g[t*128 + p]
    gh = nc.dram_tensor("gh", [128, EK], F32, kind="ExternalInput").ap()
    bvh = nc.dram_tensor("bvh", [1, E], F32, kind="ExternalInput").ap()
    out = nc.dram_tensor("out", [SQ, E], F32, kind="ExternalOutput").ap()

    with tile.TileContext(nc) as tc, ExitStack() as top:
        # ---- long-lived pools ----
        consts = top.enter_context(tc.tile_pool(name="consts", bufs=1))
        qtpool = top.enter_context(tc.tile_pool(name="qtpool", bufs=1))
        xkpool = top.enter_context(tc.tile_pool(name="xkpool", bufs=1))
        xvpool = top.enter_context(tc.tile_pool(name="xvpool", bufs=1))
        wvpool = top.enter_context(tc.tile_pool(name="wvpool", bufs=1))

        qt_tiles = [qtpool.tile([128, SQ], BF16, tag=f"qt{m}", name=f"qt{m}")
                    for m in range(EK)]
        xk_tiles = [xkpool.tile([128, S], BF16, tag=f"xk{k}", name=f"xk{k}")
                    for k in range(EK)]
        xv_tiles = [xvpool.tile([128, E], BF16, tag=f"xv{m}", name=f"xv{m}")
                    for m in range(MK)]
        wv_tiles = [wvpool.tile([128, E], BF16, tag=f"wv{k}", name=f"wv{k}")
                    for k in range(EK)]

        with tc.tile_pool(name="wqwk", bufs=1) as wqwkp, \
             tc.tile_pool(name="apool", bufs=1) as apool, \
             tc.tile_pool(name="xqpool", bufs=1) as xqpool:
            wq_t = [wqwkp.tile([128, E], BF16, tag=f"wq{c}", name=f"wq{c}")
                    for c in range(EK)]
            wk_t = [wqwkp.tile([128, E], BF16, tag=f"wk{c}", name=f"wk{c}")
                    for c in range(EK)]
            a_tiles = [apool.tile([128, E], BF16, tag=f"a{t}", name=f"a{t}")
                       for t in range(EK)]
            xq_tiles = [xqpool.tile([128, SQ], BF16, tag=f"xq{t}", name=f"xq{t}")
                        for t in range(EK)]

            # memset first: warm-up matmuls depend on it
            ones_r = consts.tile([128, 256], F32)
            nc.vector.memset(ones_r[:], 1.0)

            # ---- DMA issue order = consumption order ----
            # wq full tiles + wk first halves feed phase A's nb=0 wave
            for c in range(EK):
                nc.sync.dma_start(wq_t[c][:], wqT[c * 128:(c + 1) * 128, :])
                nc.sync.dma_start(wk_t[c][:, 0:512],
                                  wkT[c * 128:(c + 1) * 128, 0:512])
            for c in range(EK):
                nc.sync.dma_start(wk_t[c][:, 512:1024],
                                  wkT[c * 128:(c + 1) * 128, 512:1024])
            g_t = consts.tile([128, EK], F32)
            nc.sync.dma_start(g_t[:], gh)
            bv_row = consts.tile([1, E], F32)
            nc.sync.dma_start(bv_row[:], bvh)
            bv_bc = consts.tile([128, E], F32)
            nc.gpsimd.partition_broadcast(bv_bc[:], bv_row[:])
            for t in range(EK):
                nc.sync.dma_start(xq_tiles[t][:], xqT[t * 128:(t + 1) * 128, :])
            for k in range(EK):
                nc.sync.dma_start(xk_tiles[k][:], xkT[k * 128:(k + 1) * 128, :])
            for m in range(MK):
                nc.sync.dma_start(xv_tiles[m][:], xv[m * 128:(m + 1) * 128, :])
            for k in range(EK):
                nc.sync.dma_start(wv_tiles[k][:], wv[k * 128:(k + 1) * 128, :])

            # ---- PE warm-up: keep the tensor engine busy through the DMA
            # lead-in so the p-state ramp completes before real work ----
            ones_f32r = ones_r[:].bitcast(mybir.dt.float32r)
            with tc.tile_pool(name="psW", bufs=1, space="PSUM") as psW:
                warm = psW.tile([128, 256], F32, tag="warm")
                for _ in range(16):
                    nc.tensor.matmul(warm[:], ones_f32r[:, 0:128],
                                     ones_f32r, start=True, stop=True)

            # ====== phase A: A = Wq Wk^T  (c-outer PSUM waves; the final
            # half-waves let next-phase matmuls overlap the copy tail) ======
            def a_wave(nb, ts_):
                psa = {t: psA.tile([128, 512], F32, tag=f"psA{t}",
                                   name=f"psA{nb}_{t}") for t in ts_}
                for c in range(EK):
                    for t in ts_:
                        nc.tensor.matmul(
                            psa[t][:],
                            wq_t[c][:, t * 128:(t + 1) * 128],
                            wk_t[c][:, nb * 512:(nb + 1) * 512],
                            start=(c == 0), stop=(c == EK - 1))
                # drain copies split across DVE/Act
                for t in ts_:
                    dst = a_tiles[t][:, nb * 512:(nb + 1) * 512]
                    if t % 2 == 0:
                        nc.vector.tensor_scalar_add(dst, psa[t][:], 0.0)
                    else:
                        nc.scalar.copy(dst, psa[t][:])

            with tc.tile_pool(name="psA", bufs=1, space="PSUM") as psA:
                a_wave(0, range(8))
                a_wave(1, range(0, 4))
                a_wave(1, range(4, 8))

            # ====== phase Q: q''T = A^T-contraction with xqT, + g ======
            def q_wave(nb, ms_):
                psq = {m: psQ.tile([128, 512], F32, tag=f"psQ{m}",
                                   name=f"psQ{nb}_{m}") for m in ms_}
                for t in range(EK):
                    for m in ms_:
                        nc.tensor.matmul(
                            psq[m][:],
                            a_tiles[t][:, m * 128:(m + 1) * 128],
                            xq_tiles[t][:, nb * 512:(nb + 1) * 512],
                            start=(t == 0), stop=(t == EK - 1))
                for m in ms_:
                    dst = qt_tiles[m][:, nb * 512:(nb + 1) * 512]
                    if m % 2 == 0:
                        nc.vector.tensor_scalar_add(
                            dst, psq[m][:], g_t[:, m:m + 1])
                    else:
                        nc.scalar.activation(
                            dst, psq[m][:], AF.Identity,
                            bias=g_t[:, m:m + 1])

            with tc.tile_pool(name="psQ", bufs=1, space="PSUM") as psQ:
                q_wave(0, range(8))
                q_wave(1, range(0, 4))
                q_wave(1, range(4, 8))

        # ====== phase D: attention, blocked over s_q ======
        with tc.tile_pool(name="expp", bufs=1) as expp, \
             tc.tile_pool(name="ztp", bufs=2) as ztp, \
             tc.tile_pool(name="otp", bufs=1) as otp, \
             tc.tile_pool(name="partp", bufs=2) as partp, \
             tc.tile_pool(name="rcp", bufs=2) as rcp, \
             tc.tile_pool(name="psS", bufs=2, space="PSUM") as psS, \
             tc.tile_pool(name="psZ", bufs=2, space="PSUM") as psZ, \
             tc.tile_pool(name="psO", bufs=2, space="PSUM") as psO, \
             tc.tile_pool(name="psSum", bufs=1, space="PSUM") as psSum:
            for blk in range(NBLK):
                q0 = blk * BQ
                # scoresT[s_k, blk] -> exp (bf16)
                exps = []
                for m in range(MK):
                    ps = psS.tile([128, BQ], F32, tag="psS")
                    for k in range(EK):
                        nc.tensor.matmul(
                            ps[:],
                            xk_tiles[k][:, m * 128:(m + 1) * 128],
                            qt_tiles[k][:, q0:q0 + BQ],
                            start=(k == 0), stop=(k == EK - 1))
                    et = expp.tile([128, BQ], BF16, tag=f"exp{m}",
                                   name=f"exp{blk}_{m}")
                    nc.scalar.activation(et[:], ps[:], AF.Exp, scale=INV_SCALE)
                    exps.append(et)

                # partial sums over s_k tiles (DVE chain), overlaps ZT below
                part = partp.tile([128, BQ], F32, tag="part",
                                  name=f"part{blk}")
                nc.vector.tensor_add(part[:], exps[0][:], exps[1][:])
                for m in range(2, MK - 1):
                    nc.vector.tensor_add(part[:], part[:], exps[m][:])
                part_r = partp.tile([128, BQ], mybir.dt.float32r, tag="part_r",
                                    name=f"part_r{blk}")
                nc.vector.tensor_add(part_r[:], part[:], exps[MK - 1][:])

                # Z^T[e, i] = sum_j Xv[j, e] expT[j, i]
                zts = []
                for e_ in range(EK):
                    ps = psZ.tile([128, BQ], F32, tag="psZ")
                    for m in range(MK):
                        nc.tensor.matmul(
                            ps[:],
                            xv_tiles[m][:, e_ * 128:(e_ + 1) * 128],
                            exps[m][:],
                            start=(m == 0), stop=(m == MK - 1))
                    zt = ztp.tile([128, BQ], BF16, tag=f"zt{e_}",
                                  name=f"zt{blk}_{e_}")
                    nc.scalar.copy(zt[:], ps[:])
                    zts.append(zt)

                # partition-reduce of part_r via 256-wide ones-matmuls
                # (real-size matmuls keep the PE p-state stretch alive)
                pssum = psSum.tile([128, 256 * (BQ // 128)], F32, tag="psSum")
                recips = []
                for s in range(BQ // 128):
                    nc.tensor.matmul(
                        pssum[:, s * 256:(s + 1) * 256],
                        part_r[:, s * 128:(s + 1) * 128],
                        ones_r[:].bitcast(mybir.dt.float32r),
                        start=True, stop=True)
                    rc = rcp.tile([128, 1], F32, tag=f"rc{s}",
                                  name=f"rc{blk}_{s}")
                    nc.vector.reciprocal(rc[:], pssum[:, s * 256:s * 256 + 1])
                    recips.append(rc)

                # O = Z @ Wv, normalize by recip, + bv, DMA out
                for it in range(BQ // 128):
                    ot = otp.tile([128, E], F32, tag=f"ot{it}",
                                  name=f"ot{blk}_{it}")
                    for n in range(E // 512):
                        ps = psO.tile([128, 512], F32, tag="psO")
                        for e_ in range(EK):
                            nc.tensor.matmul(
                                ps[:],
                                zts[e_][:, it * 128:(it + 1) * 128],
                                wv_tiles[e_][:, n * 512:(n + 1) * 512],
                                start=(e_ == 0), stop=(e_ == EK - 1))
                        nc.scalar.activation(
                            ot[:, n * 512:(n + 1) * 512], ps[:],
                            AF.Copy, scale=recips[it][:])
                        nc.vector.tensor_add(
                            ot[:, n * 512:(n + 1) * 512],
                            ot[:, n * 512:(n + 1) * 512],
                            bv_bc[:, n * 512:(n + 1) * 512])
                        # per-half DMA keeps the end-of-kernel tail short
                        nc.sync.dma_start(
                            out[q0 + it * 128:q0 + (it + 1) * 128,
                                n * 512:(n + 1) * 512],
                            ot[:, n * 512:(n + 1) * 512])

    nc.compile()
    return nc


def _get_nc():
    if "nc" not in _cached:
        _cached["nc"] = _build()
    return _cached["nc"]


def _bf16(a):
    return np.ascontiguousarray(np.asarray(a, dtype=np.float32)).astype(
        ml_dtypes.bfloat16)


def kernel(query, key, value, Wq, bq, Wk, bk, Wv, bv, **kw):
    query = np.asarray(query, dtype=np.float32)
    key = np.asarray(key, dtype=np.float32)
    value = np.asarray(value, dtype=np.float32)
    Wq = np.asarray(Wq, dtype=np.float32)
    Wk = np.asarray(Wk, dtype=np.float32)
    Wv = np.asarray(Wv, dtype=np.float32)
    bq = np.asarray(bq, dtype=np.float32)
    bv = np.asarray(bv, dtype=np.float32)

    wqT_h = _bf16(Wq.T)
    wkT_h = _bf16(Wk.T)
    wv_h = _bf16(Wv)
    g = Wk @ bq                       # [E]; bk cancels in softmax
    g_h = np.ascontiguousarray(g.reshape(EK, 128).T).astype(np.float32)
    bv_h = np.ascontiguousarray(bv.reshape(1, E))

    keyT = {b: _bf16(key[b].T) for b in range(B)}
    valN = {b: _bf16(value[b]) for b in range(B)}

    in_maps = []
    for c in range(N_CORES):
        b, h = divmod(c, 2)
        qT = _bf16(query[b, h * SQ:(h + 1) * SQ, :].T)
        in_maps.append({
            "wqT": wqT_h, "wkT": wkT_h, "xqT": qT,
            "xkT": keyT[b], "xv": valN[b], "wv": wv_h,
            "gh": g_h, "bvh": bv_h,
        })

    nc = _get_nc()
    res = bass_utils.run_bass_kernel_spmd(
        nc, in_maps, core_ids=list(range(N_CORES)), **kw)

    full = np.empty((B, S, E), dtype=np.float32)
    for c in range(N_CORES):
        b, h = divmod(c, 2)
        full[b, h * SQ:(h + 1) * SQ, :] = res.results[c]["out"]
    kernel.last_results = res
    return full


# revision 16
# speedup vs baseline: 1.5427x; 1.0093x over previous
"""Trainium2 Bass kernel for single-head attention model.

Reference computation (B=4, S=2048, E=1024, fp32):
    q = query @ Wq + bq;  k = key @ Wk + bk;  v = value @ Wv + bv
    scores = (q @ k^T) / sqrt(E)
    out = softmax(scores, axis=-1) @ v

Sharding: 8 cores; core c handles batch b = c // 2, query-row half
h = c % 2 (1024 q-rows). No collectives.

Algebraic restructure (saves ~23% of the MACs vs the direct form):
    scores_ij = x^q_i A x^k_j + g.x^k_j (+ row-const terms that cancel
    in softmax), where A = Wq Wk^T and g = Wk bq (host-computed).
    bk drops out entirely.  On the value side,
    out = softmax(scores) @ (Xv Wv + bv) = (attn @ Xv) @ Wv + bv
    since attn rows sum to 1 — Wv is applied to only the core's own
    1024 q rows instead of all 2048 kv rows.

Per-core matmul work (128x128 PE, 1 cycle/row at free>=256):
    A = WqWk^T (65536 cyc) ; q'T = A^T-contract with xqT (65536)
    scoresT (131072) ; Z^T = Xv^T-contract with exp (131072)
    O = Z Wv (65536)  => 458752 cycles ~= 191us @2.4GHz.

All matmul inputs are bf16 (host-converted); PSUM accumulates f32.
exp/Z intermediates stored bf16.  Softmax sums come from tiny
ones-matmuls accumulated over the 16 key tiles.
"""

import sys

sys.path.insert(0, "/opt/trn_rl_repo")

from contextlib import ExitStack

import ml_dtypes
import numpy as np

import concourse.bass as bass
import concourse.mybir as mybir
import concourse.tile as tile
from concourse import bacc, bass_utils

BF16 = mybir.dt.bfloat16
F32 = mybir.dt.float32
AF = mybir.ActivationFunctionType

B, S, E = 4, 2048, 1024
N_CORES = 8
SQ = S // 2          # q rows per core
BQ = 512             # s_q block width in attention phase
NBLK = SQ // BQ      # 2 blocks
EK = E // 128        # 8 tiles over e/a/c dims
MK = S // 128        # 16 s_k tiles
INV_SCALE = 1.0 / float(np.sqrt(E))

_cached = {}


def _build():
    nc = bacc.Bacc("TRN2", target_bir_lowering=False, debug=False,
                   num_devices=N_CORES)

    # host pre-transposed / pre-converted inputs (all bf16 except consts)
    wqT = nc.dram_tensor("wqT", [E, E], BF16, kind="ExternalInput").ap()
    wkT = nc.dram_tensor("wkT", [E, E], BF16, kind="ExternalInput").ap()
    xqT = nc.dram_tensor("xqT", [E, SQ], BF16, kind="ExternalInput").ap()
    xkT = nc.dram_tensor("xkT", [E, S], BF16, kind="ExternalInput").ap()
    xv = nc.dram_tensor("xv", [S, E], BF16, kind="ExternalInput").ap()
    wv = nc.dram_tensor("wv", [E, E], BF16, kind="ExternalInput").ap()
    # g = Wk @ bq arranged g_h[p, t] = g[t*128 + p]
    gh = nc.dram_tensor("gh", [128, EK], F32, kind="ExternalInput").ap()
    bvh = nc.dram_tensor("bvh", [1, E], F32, kind="ExternalInput").ap()
    out = nc.dram_tensor("out", [SQ, E], F32, kind="ExternalOutput").ap()

    with tile.TileContext(nc) as tc, ExitStack() as top:
        # ---- long-lived pools ----
        consts = top.enter_context(tc.tile_pool(name="consts", bufs=1))
        qtpool = top.enter_context(tc.tile_pool(name="qtpool", bufs=1))
        xkpool = top.enter_context(tc.tile_pool(name="xkpool", bufs=1))
        xvpool = top.enter_context(tc.tile_pool(name="xvpool", bufs=1))
        wvpool = top.enter_context(tc.tile_pool(name="wvpool", bufs=1))

        # single shared PSUM pool: 8 tags x [128,512]f32 = 8 banks; shared
        # tags across phases avoid pool release/alloc barriers entirely
        psp = top.enter_context(tc.tile_pool(name="psp", bufs=1, space="PSUM"))

        qt_tiles = [qtpool.tile([128, SQ], BF16, tag=f"qt{m}", name=f"qt{m}")
                    for m in range(EK)]
        xk_tiles = [xkpool.tile([128, S], BF16, tag=f"xk{k}", name=f"xk{k}")
                    for k in range(EK)]
        xv_tiles = [xvpool.tile([128, E], BF16, tag=f"xv{m}", name=f"xv{m}")
                    for m in range(MK)]
        wv_tiles = [wvpool.tile([128, E], BF16, tag=f"wv{k}", name=f"wv{k}")
                    for k in range(EK)]

        with tc.tile_pool(name="wqwk", bufs=1) as wqwkp, \
             tc.tile_pool(name="apool", bufs=1) as apool, \
             tc.tile_pool(name="xqpool", bufs=1) as xqpool:
            wq_t = [wqwkp.tile([128, E], BF16, tag=f"wq{c}", name=f"wq{c}")
                    for c in range(EK)]
            wk_t = [wqwkp.tile([128, E], BF16, tag=f"wk{c}", name=f"wk{c}")
                    for c in range(EK)]
            a_tiles = [apool.tile([128, E], BF16, tag=f"a{t}", name=f"a{t}")
                       for t in range(EK)]
            xq_tiles = [xqpool.tile([128, SQ], BF16, tag=f"xq{t}", name=f"xq{t}")
                        for t in range(EK)]

            # memset first: warm-up matmuls depend on it
            ones_r = consts.tile([128, 256], F32)
            nc.vector.memset(ones_r[:], 1.0)

            # ---- DMA issue order = consumption order ----
            # wq full tiles + wk first halves feed phase A's nb=0 wave
            for c in range(EK):
                nc.sync.dma_start(wq_t[c][:], wqT[c * 128:(c + 1) * 128, :])
                nc.sync.dma_start(wk_t[c][:, 0:512],
                                  wkT[c * 128:(c + 1) * 128, 0:512])
            for c in range(EK):
                nc.sync.dma_start(wk_t[c][:, 512:1024],
                                  wkT[c * 128:(c + 1) * 128, 512:1024])
            g_t = consts.tile([128, EK], F32)
            nc.sync.dma_start(g_t[:], gh)
            bv_row = consts.tile([1, E], F32)
            nc.sync.dma_start(bv_row[:], bvh)
            bv_bc = consts.tile([128, E], F32)
            nc.gpsimd.partition_broadcast(bv_bc[:], bv_row[:])
            for t in range(EK):
                nc.sync.dma_start(xq_tiles[t][:], xqT[t * 128:(t + 1) * 128, :])
            for k in range(EK):
                nc.sync.dma_start(xk_tiles[k][:], xkT[k * 128:(k + 1) * 128, :])
            for m in range(MK):
                nc.sync.dma_start(xv_tiles[m][:], xv[m * 128:(m + 1) * 128, :])
            for k in range(EK):
                nc.sync.dma_start(wv_tiles[k][:], wv[k * 128:(k + 1) * 128, :])

            # ---- PE warm-up: keep the tensor engine busy through the DMA
            # lead-in so the p-state ramp completes before real work ----
            ones_f32r = ones_r[:].bitcast(mybir.dt.float32r)
            warm = psp.tile([128, 256], F32, tag="ps0", name="warm")
            for _ in range(16):
                nc.tensor.matmul(warm[:], ones_f32r[:, 0:128],
                                 ones_f32r, start=True, stop=True)

            # ====== phase A: A = Wq Wk^T  (c-outer PSUM waves; the final
            # half-waves let next-phase matmuls overlap the copy tail) ======
            def a_wave(nb, ts_):
                psa = {t: psp.tile([128, 512], F32, tag=f"ps{t}",
                                   name=f"psA{nb}_{t}") for t in ts_}
                for c in range(EK):
                    for t in ts_:
                        nc.tensor.matmul(
                            psa[t][:],
                            wq_t[c][:, t * 128:(t + 1) * 128],
                            wk_t[c][:, nb * 512:(nb + 1) * 512],
                            start=(c == 0), stop=(c == EK - 1))
                # drain copies split across DVE/Act
                for t in ts_:
                    dst = a_tiles[t][:, nb * 512:(nb + 1) * 512]
                    if t % 2 == 0:
                        nc.vector.tensor_scalar_add(dst, psa[t][:], 0.0)
                    else:
                        nc.scalar.copy(dst, psa[t][:])

            a_wave(0, range(8))
            a_wave(1, range(0, 4))
            a_wave(1, range(4, 8))

            # ====== phase Q: q''T = A^T-contraction with xqT, + g ======
            def q_wave(nb, ms_):
                psq = {m: psp.tile([128, 512], F32, tag=f"ps{m}",
                                   name=f"psQ{nb}_{m}") for m in ms_}
                for t in range(EK):
                    for m in ms_:
                        nc.tensor.matmul(
                            psq[m][:],
                            a_tiles[t][:, m * 128:(m + 1) * 128],
                            xq_tiles[t][:, nb * 512:(nb + 1) * 512],
                            start=(t == 0), stop=(t == EK - 1))
                for m in ms_:
                    dst = qt_tiles[m][:, nb * 512:(nb + 1) * 512]
                    if m % 2 == 0:
                        nc.vector.tensor_scalar_add(
                            dst, psq[m][:], g_t[:, m:m + 1])
                    else:
                        nc.scalar.activation(
                            dst, psq[m][:], AF.Identity,
                            bias=g_t[:, m:m + 1])

            q_wave(0, range(8))
            q_wave(1, range(0, 4))
            q_wave(1, range(4, 8))

        # ====== phase D: attention, blocked over s_q ======
        with tc.tile_pool(name="expp", bufs=1) as expp, \
             tc.tile_pool(name="ztp", bufs=2) as ztp, \
             tc.tile_pool(name="otp", bufs=1) as otp, \
             tc.tile_pool(name="partp", bufs=2) as partp, \
             tc.tile_pool(name="rcp", bufs=2) as rcp:
            for blk in range(NBLK):
                q0 = blk * BQ
                # scoresT[s_k, blk] -> exp (bf16)
                exps = []
                for m in range(MK):
                    ps = psp.tile([128, BQ], F32, tag=f"ps{m % 2}",
                                  name=f"psS{blk}_{m}")
                    for k in range(EK):
                        nc.tensor.matmul(
                            ps[:],
                            xk_tiles[k][:, m * 128:(m + 1) * 128],
                            qt_tiles[k][:, q0:q0 + BQ],
                            start=(k == 0), stop=(k == EK - 1))
                    et = expp.tile([128, BQ], BF16, tag=f"exp{m}",
                                   name=f"exp{blk}_{m}")
                    nc.scalar.activation(et[:], ps[:], AF.Exp, scale=INV_SCALE)
                    exps.append(et)

                # partial sums over s_k tiles (DVE chain), overlaps ZT below
                part = partp.tile([128, BQ], F32, tag="part",
                                  name=f"part{blk}")
                nc.vector.tensor_add(part[:], exps[0][:], exps[1][:])
                for m in range(2, MK - 1):
                    nc.vector.tensor_add(part[:], part[:], exps[m][:])
                part_r = partp.tile([128, BQ], mybir.dt.float32r, tag="part_r",
                                    name=f"part_r{blk}")
                nc.vector.tensor_add(part_r[:], part[:], exps[MK - 1][:])

                # Z^T[e, i] = sum_j Xv[j, e] expT[j, i]
                zts = []
                for e_ in range(EK):
                    ps = psp.tile([128, BQ], F32, tag=f"ps{2 + e_ % 2}",
                                  name=f"psZ{blk}_{e_}")
                    for m in range(MK):
                        nc.tensor.matmul(
                            ps[:],
                            xv_tiles[m][:, e_ * 128:(e_ + 1) * 128],
                            exps[m][:],
                            start=(m == 0), stop=(m == MK - 1))
                    zt = ztp.tile([128, BQ], BF16, tag=f"zt{e_}",
                                  name=f"zt{blk}_{e_}")
                    nc.scalar.copy(zt[:], ps[:])
                    zts.append(zt)

                # partition-reduce of part_r via 256-wide ones-matmuls
                # (real-size matmuls keep the PE p-state stretch alive)
                recips = []
                for sh in range(2):
                    pssum = psp.tile([128, 512], F32, tag=f"ps{6 + sh}",
                                     name=f"psSum{blk}_{sh}")
                    for sl in range(2):
                        s = sh * 2 + sl
                        nc.tensor.matmul(
                            pssum[:, sl * 256:(sl + 1) * 256],
                            part_r[:, s * 128:(s + 1) * 128],
                            ones_r[:].bitcast(mybir.dt.float32r),
                            start=True, stop=True)
                    for sl in range(2):
                        s = sh * 2 + sl
                        rc = rcp.tile([128, 1], F32, tag=f"rc{s}",
                                      name=f"rc{blk}_{s}")
                        nc.vector.reciprocal(
                            rc[:], pssum[:, sl * 256:sl * 256 + 1])
                        recips.append(rc)

                # O = Z @ Wv, normalize by recip, + bv, DMA out
                for it in range(BQ // 128):
                    ot = otp.tile([128, E], F32, tag=f"ot{it}",
                                  name=f"ot{blk}_{it}")
                    last_it = (blk == NBLK - 1 and it == BQ // 128 - 1)
                    for n in range(E // 512):
                        ps = psp.tile([128, 512], F32, tag=f"ps{4 + n}",
                                      name=f"psO{blk}_{it}_{n}")
                        for e_ in range(EK):
                            nc.tensor.matmul(
                                ps[:],
                                zts[e_][:, it * 128:(it + 1) * 128],
                                wv_tiles[e_][:, n * 512:(n + 1) * 512],
                                start=(e_ == 0), stop=(e_ == EK - 1))
                        # finer chunks on the very last group shorten the
                        # end-of-kernel tail; per-half DMA elsewhere
                        cw = 256 if (last_it and n == E // 512 - 1) else 512
                        for f0 in range(n * 512, (n + 1) * 512, cw):
                            nc.scalar.activation(
                                ot[:, f0:f0 + cw], ps[:, f0 - n * 512:
                                                      f0 - n * 512 + cw],
                                AF.Copy, scale=recips[it][:])
                            nc.vector.tensor_add(
                                ot[:, f0:f0 + cw],
                                ot[:, f0:f0 + cw],
                                bv_bc[:, f0:f0 + cw])
                            nc.sync.dma_start(
                                out[q0 + it * 128:q0 + (it + 1) * 128,
                                    f0:f0 + cw],
                                ot[:, f0:f0 + cw])

    nc.compile()
    return nc


def _get_nc():
    if "nc" not in _cached:
        _cached["nc"] = _build()
    return _cached["nc"]


def _bf16(a):
    return np.ascontiguousarray(np.asarray(a, dtype=np.float32)).astype(
        ml_dtypes.bfloat16)


def kernel(query, key, value, Wq, bq, Wk, bk, Wv, bv, **kw):
    query = np.asarray(query, dtype=np.float32)
    key = np.asarray(key, dtype=np.float32)
    value = np.asarray(value, dtype=np.float32)
    Wq = np.asarray(Wq, dtype=np.float32)
    Wk = np.asarray(Wk, dtype=np.float32)
    Wv = np.asarray(Wv, dtype=np.float32)
    bq = np.asarray(bq, dtype=np.float32)
    bv = np.asarray(bv, dtype=np.float32)

    wqT_h = _bf16(Wq.T)
    wkT_h = _bf16(Wk.T)
    wv_h = _bf16(Wv)
    g = Wk @ bq                       # [E]; bk cancels in softmax
    g_h = np.ascontiguousarray(g.reshape(EK, 128).T).astype(np.float32)
    bv_h = np.ascontiguousarray(bv.reshape(1, E))

    keyT = {b: _bf16(key[b].T) for b in range(B)}
    valN = {b: _bf16(value[b]) for b in range(B)}

    in_maps = []
    for c in range(N_CORES):
        b, h = divmod(c, 2)
        qT = _bf16(query[b, h * SQ:(h + 1) * SQ, :].T)
        in_maps.append({
            "wqT": wqT_h, "wkT": wkT_h, "xqT": qT,
            "xkT": keyT[b], "xv": valN[b], "wv": wv_h,
            "gh": g_h, "bvh": bv_h,
        })

    nc = _get_nc()
    res = bass_utils.run_bass_kernel_spmd(
        nc, in_maps, core_ids=list(range(N_CORES)), **kw)

    full = np.empty((B, S, E), dtype=np.float32)
    for c in range(N_CORES):
        b, h = divmod(c, 2)
        full[b, h * SQ:(h + 1) * SQ, :] = res.results[c]["out"]
    kernel.last_results = res
    return full


# revision 19
# speedup vs baseline: 1.5557x; 1.0084x over previous
"""Trainium2 Bass kernel for single-head attention model.

Reference computation (B=4, S=2048, E=1024, fp32):
    q = query @ Wq + bq;  k = key @ Wk + bk;  v = value @ Wv + bv
    scores = (q @ k^T) / sqrt(E)
    out = softmax(scores, axis=-1) @ v

Sharding: 8 cores; core c handles batch b = c // 2, query-row half
h = c % 2 (1024 q-rows). No collectives.

Algebraic restructure (saves ~23% of the MACs vs the direct form):
    scores_ij = x^q_i A x^k_j + g.x^k_j (+ row-const terms that cancel
    in softmax), where A = Wq Wk^T and g = Wk bq (host-computed).
    bk drops out entirely.  On the value side,
    out = softmax(scores) @ (Xv Wv + bv) = (attn @ Xv) @ Wv + bv
    since attn rows sum to 1 — Wv is applied to only the core's own
    1024 q rows instead of all 2048 kv rows.

Per-core matmul work (128x128 PE, 1 cycle/row at free>=256):
    A = WqWk^T (65536 cyc) ; q'T = A^T-contract with xqT (65536)
    scoresT (131072) ; Z^T = Xv^T-contract with exp (131072)
    O = Z Wv (65536)  => 458752 cycles ~= 191us @2.4GHz.

All matmul inputs are bf16 (host-converted); PSUM accumulates f32.
exp/Z intermediates stored bf16.  Softmax sums come from tiny
ones-matmuls accumulated over the 16 key tiles.
"""

import sys

sys.path.insert(0, "/opt/trn_rl_repo")

from contextlib import ExitStack

import ml_dtypes
import numpy as np

import concourse.bass as bass
import concourse.mybir as mybir
import concourse.tile as tile
from concourse import bacc, bass_utils

BF16 = mybir.dt.bfloat16
F32 = mybir.dt.float32
AF = mybir.ActivationFunctionType

B, S, E = 4, 2048, 1024
N_CORES = 8
SQ = S // 2          # q rows per core
BQ = 512             # s_q block width in attention phase
NBLK = SQ // BQ      # 2 blocks
EK = E // 128        # 8 tiles over e/a/c dims
MK = S // 128        # 16 s_k tiles
INV_SCALE = 1.0 / float(np.sqrt(E))

_cached = {}


def _build():
    nc = bacc.Bacc("TRN2", target_bir_lowering=False, debug=False,
                   num_devices=N_CORES)

    # host pre-transposed / pre-converted inputs (all bf16 except consts)
    wqT = nc.dram_tensor("wqT", [E, E], BF16, kind="ExternalInput").ap()
    wkT = nc.dram_tensor("wkT", [E, E], BF16, kind="ExternalInput").ap()
    xqT = nc.dram_tensor("xqT", [E, SQ], BF16, kind="ExternalInput").ap()
    xkT = nc.dram_tensor("xkT", [E, S], BF16, kind="ExternalInput").ap()
    xv = nc.dram_tensor("xv", [S, E], BF16, kind="ExternalInput").ap()
    wv = nc.dram_tensor("wv", [E, E], BF16, kind="ExternalInput").ap()
    # g = Wk @ bq arranged g_h[p, t] = g[t*128 + p]
    gh = nc.dram_tensor("gh", [128, EK], F32, kind="ExternalInput").ap()
    bvh = nc.dram_tensor("bvh", [1, E], F32, kind="ExternalInput").ap()
    out = nc.dram_tensor("out", [SQ, E], F32, kind="ExternalOutput").ap()

    with tile.TileContext(nc) as tc, ExitStack() as top:
        # ---- long-lived pools ----
        consts = top.enter_context(tc.tile_pool(name="consts", bufs=1))
        qtpool = top.enter_context(tc.tile_pool(name="qtpool", bufs=1))
        xkpool = top.enter_context(tc.tile_pool(name="xkpool", bufs=1))
        xvpool = top.enter_context(tc.tile_pool(name="xvpool", bufs=1))
        wvpool = top.enter_context(tc.tile_pool(name="wvpool", bufs=1))

        # single shared PSUM pool: 8 tags x [128,512]f32 = 8 banks; shared
        # tags across phases avoid pool release/alloc barriers entirely
        psp = top.enter_context(tc.tile_pool(name="psp", bufs=1, space="PSUM"))

        qt_tiles = [qtpool.tile([128, SQ], BF16, tag=f"qt{m}", name=f"qt{m}")
                    for m in range(EK)]
        xk_tiles = [xkpool.tile([128, S], BF16, tag=f"xk{k}", name=f"xk{k}")
                    for k in range(EK)]
        xv_tiles = [xvpool.tile([128, E], BF16, tag=f"xv{m}", name=f"xv{m}")
                    for m in range(MK)]
        wv_tiles = [wvpool.tile([128, E], BF16, tag=f"wv{k}", name=f"wv{k}")
                    for k in range(EK)]

        with tc.tile_pool(name="wqwk", bufs=1) as wqwkp, \
             tc.tile_pool(name="apool", bufs=1) as apool, \
             tc.tile_pool(name="xqpool", bufs=1) as xqpool:
            wq_t = [wqwkp.tile([128, E], BF16, tag=f"wq{c}", name=f"wq{c}")
                    for c in range(EK)]
            wk_t = [wqwkp.tile([128, E], BF16, tag=f"wk{c}", name=f"wk{c}")
                    for c in range(EK)]
            a_tiles = [apool.tile([128, E], BF16, tag=f"a{t}", name=f"a{t}")
                       for t in range(EK)]
            xq_tiles = [xqpool.tile([128, SQ], BF16, tag=f"xq{t}", name=f"xq{t}")
                        for t in range(EK)]

            ones_r = consts.tile([128, 256], F32)
            ones_f32r = ones_r[:].bitcast(mybir.dt.float32r)

            # ---- PE warm-up: keep the tensor engine busy through the DMA
            # lead-in so the p-state ramp completes before real work.
            # Reads ones_r UNINITIALIZED on purpose (values never consumed);
            # the memset below is WAR-ordered after the warm-up reads and
            # completes long before the sums-matmuls need real ones. ----
            warm = psp.tile([128, 256], F32, tag="ps0", name="warm")
            for _ in range(14):
                nc.tensor.matmul(warm[:], ones_f32r[:, 0:128],
                                 ones_f32r, start=True, stop=True)
            nc.vector.memset(ones_r[:], 1.0)

            # ---- DMA issue order = consumption order ----
            # wq full tiles + wk first halves feed phase A's nb=0 wave
            for c in range(EK):
                nc.sync.dma_start(wq_t[c][:], wqT[c * 128:(c + 1) * 128, :])
                nc.sync.dma_start(wk_t[c][:, 0:512],
                                  wkT[c * 128:(c + 1) * 128, 0:512])
            for c in range(EK):
                nc.sync.dma_start(wk_t[c][:, 512:1024],
                                  wkT[c * 128:(c + 1) * 128, 512:1024])
            g_t = consts.tile([128, EK], F32)
            nc.sync.dma_start(g_t[:], gh)
            bv_row = consts.tile([1, E], F32)
            nc.sync.dma_start(bv_row[:], bvh)
            bv_bc = consts.tile([128, E], F32)
            nc.gpsimd.partition_broadcast(bv_bc[:], bv_row[:])
            for t in range(EK):
                nc.sync.dma_start(xq_tiles[t][:], xqT[t * 128:(t + 1) * 128, :])
            for k in range(EK):
                nc.sync.dma_start(xk_tiles[k][:], xkT[k * 128:(k + 1) * 128, :])
            for m in range(MK):
                nc.sync.dma_start(xv_tiles[m][:], xv[m * 128:(m + 1) * 128, :])
            for k in range(EK):
                nc.sync.dma_start(wv_tiles[k][:], wv[k * 128:(k + 1) * 128, :])

            # ====== phase A: A = Wq Wk^T  (c-outer PSUM waves; the final
            # half-waves let next-phase matmuls overlap the copy tail) ======
            def a_wave(nb, ts_):
                psa = {t: psp.tile([128, 512], F32, tag=f"ps{t}",
                                   name=f"psA{nb}_{t}") for t in ts_}
                for c in range(EK):
                    for t in ts_:
                        nc.tensor.matmul(
                            psa[t][:],
                            wq_t[c][:, t * 128:(t + 1) * 128],
                            wk_t[c][:, nb * 512:(nb + 1) * 512],
                            start=(c == 0), stop=(c == EK - 1))
                # drain copies split across DVE/Act
                for t in ts_:
                    dst = a_tiles[t][:, nb * 512:(nb + 1) * 512]
                    if t % 2 == 0:
                        nc.vector.tensor_scalar_add(dst, psa[t][:], 0.0)
                    else:
                        nc.scalar.copy(dst, psa[t][:])

            a_wave(0, range(8))
            a_wave(1, range(0, 4))
            a_wave(1, range(4, 8))

            # ====== phase Q: q''T = A^T-contraction with xqT, + g ======
            def q_wave(nb, ms_):
                psq = {m: psp.tile([128, 512], F32, tag=f"ps{m}",
                                   name=f"psQ{nb}_{m}") for m in ms_}
                for t in range(EK):
                    for m in ms_:
                        nc.tensor.matmul(
                            psq[m][:],
                            a_tiles[t][:, m * 128:(m + 1) * 128],
                            xq_tiles[t][:, nb * 512:(nb + 1) * 512],
                            start=(t == 0), stop=(t == EK - 1))
                for m in ms_:
                    dst = qt_tiles[m][:, nb * 512:(nb + 1) * 512]
                    if m % 2 == 0:
                        nc.vector.tensor_scalar_add(
                            dst, psq[m][:], g_t[:, m:m + 1])
                    else:
                        nc.scalar.activation(
                            dst, psq[m][:], AF.Identity,
                            bias=g_t[:, m:m + 1])

            q_wave(0, range(8))
            q_wave(1, range(0, 4))
            q_wave(1, range(4, 8))

        # ====== phase D: attention, blocked over s_q ======
        with tc.tile_pool(name="expp", bufs=1) as expp, \
             tc.tile_pool(name="ztp", bufs=2) as ztp, \
             tc.tile_pool(name="otp", bufs=1) as otp, \
             tc.tile_pool(name="partp", bufs=2) as partp, \
             tc.tile_pool(name="rcp", bufs=2) as rcp:
            for blk in range(NBLK):
                q0 = blk * BQ
                # scoresT[s_k, blk] -> exp (bf16)
                exps = []
                for m in range(MK):
                    ps = psp.tile([128, BQ], F32, tag=f"ps{m % 2}",
                                  name=f"psS{blk}_{m}")
                    for k in range(EK):
                        nc.tensor.matmul(
                            ps[:],
                            xk_tiles[k][:, m * 128:(m + 1) * 128],
                            qt_tiles[k][:, q0:q0 + BQ],
                            start=(k == 0), stop=(k == EK - 1))
                    et = expp.tile([128, BQ], BF16, tag=f"exp{m}",
                                   name=f"exp{blk}_{m}")
                    nc.scalar.activation(et[:], ps[:], AF.Exp, scale=INV_SCALE)
                    exps.append(et)

                # partial sums over s_k tiles (DVE chain), overlaps ZT below
                part = partp.tile([128, BQ], F32, tag="part",
                                  name=f"part{blk}")
                nc.vector.tensor_add(part[:], exps[0][:], exps[1][:])
                for m in range(2, MK - 1):
                    nc.vector.tensor_add(part[:], part[:], exps[m][:])
                part_r = partp.tile([128, BQ], mybir.dt.float32r, tag="part_r",
                                    name=f"part_r{blk}")
                nc.vector.tensor_add(part_r[:], part[:], exps[MK - 1][:])

                # Z^T[e, i] = sum_j Xv[j, e] expT[j, i]
                zts = []
                for e_ in range(EK):
                    ps = psp.tile([128, BQ], F32, tag=f"ps{2 + e_ % 2}",
                                  name=f"psZ{blk}_{e_}")
                    for m in range(MK):
                        nc.tensor.matmul(
                            ps[:],
                            xv_tiles[m][:, e_ * 128:(e_ + 1) * 128],
                            exps[m][:],
                            start=(m == 0), stop=(m == MK - 1))
                    zt = ztp.tile([128, BQ], BF16, tag=f"zt{e_}",
                                  name=f"zt{blk}_{e_}")
                    nc.scalar.copy(zt[:], ps[:])
                    zts.append(zt)

                # partition-reduce of part_r via 256-wide ones-matmuls
                # (real-size matmuls keep the PE p-state stretch alive)
                recips = []
                for sh in range(2):
                    pssum = psp.tile([128, 512], F32, tag=f"ps{6 + sh}",
                                     name=f"psSum{blk}_{sh}")
                    for sl in range(2):
                        s = sh * 2 + sl
                        nc.tensor.matmul(
                            pssum[:, sl * 256:(sl + 1) * 256],
                            part_r[:, s * 128:(s + 1) * 128],
                            ones_r[:].bitcast(mybir.dt.float32r),
                            start=True, stop=True)
                    for sl in range(2):
                        s = sh * 2 + sl
                        rc = rcp.tile([128, 1], F32, tag=f"rc{s}",
                                      name=f"rc{blk}_{s}")
                        nc.vector.reciprocal(
                            rc[:], pssum[:, sl * 256:sl * 256 + 1])
                        recips.append(rc)

                # O = Z @ Wv, normalize by recip, + bv, DMA out.
                # The very last i-tile uses 256-wide PSUM groups so its
                # post-processing + DMAs overlap the final matmuls.
                for it in range(BQ // 128):
                    ot = otp.tile([128, E], F32, tag=f"ot{it}",
                                  name=f"ot{blk}_{it}")
                    last_it = (blk == NBLK - 1 and it == BQ // 128 - 1)
                    cw = 256 if last_it else 512
                    for ci, f0 in enumerate(range(0, E, cw)):
                        ps = psp.tile([128, cw], F32, tag=f"ps{4 + ci % 2}",
                                      name=f"psO{blk}_{it}_{ci}")
                        for e_ in range(EK):
                            nc.tensor.matmul(
                                ps[:],
                                zts[e_][:, it * 128:(it + 1) * 128],
                                wv_tiles[e_][:, f0:f0 + cw],
                                start=(e_ == 0), stop=(e_ == EK - 1))
                        nc.scalar.activation(
                            ot[:, f0:f0 + cw], ps[:],
                            AF.Copy, scale=recips[it][:])
                        nc.vector.tensor_add(
                            ot[:, f0:f0 + cw],
                            ot[:, f0:f0 + cw],
                            bv_bc[:, f0:f0 + cw])
                        nc.sync.dma_start(
                            out[q0 + it * 128:q0 + (it + 1) * 128,
                                f0:f0 + cw],
                            ot[:, f0:f0 + cw])

    nc.compile()
    return nc


def _get_nc():
    if "nc" not in _cached:
        _cached["nc"] = _build()
    return _cached["nc"]


def _bf16(a):
    return np.ascontiguousarray(np.asarray(a, dtype=np.float32)).astype(
        ml_dtypes.bfloat16)


def kernel(query, key, value, Wq, bq, Wk, bk, Wv, bv, **kw):
    query = np.asarray(query, dtype=np.float32)
    key = np.asarray(key, dtype=np.float32)
    value = np.asarray(value, dtype=np.float32)
    Wq = np.asarray(Wq, dtype=np.float32)
    Wk = np.asarray(Wk, dtype=np.float32)
    Wv = np.asarray(Wv, dtype=np.float32)
    bq = np.asarray(bq, dtype=np.float32)
    bv = np.asarray(bv, dtype=np.float32)

    wqT_h = _bf16(Wq.T)
    wkT_h = _bf16(Wk.T)
    wv_h = _bf16(Wv)
    g = Wk @ bq                       # [E]; bk cancels in softmax
    g_h = np.ascontiguousarray(g.reshape(EK, 128).T).astype(np.float32)
    bv_h = np.ascontiguousarray(bv.reshape(1, E))

    keyT = {b: _bf16(key[b].T) for b in range(B)}
    valN = {b: _bf16(value[b]) for b in range(B)}

    in_maps = []
    for c in range(N_CORES):
        b, h = divmod(c, 2)
        qT = _bf16(query[b, h * SQ:(h + 1) * SQ, :].T)
        in_maps.append({
            "wqT": wqT_h, "wkT": wkT_h, "xqT": qT,
            "xkT": keyT[b], "xv": valN[b], "wv": wv_h,
            "gh": g_h, "bvh": bv_h,
        })

    nc = _get_nc()
    res = bass_utils.run_bass_kernel_spmd(
        nc, in_maps, core_ids=list(range(N_CORES)), **kw)

    full = np.empty((B, S, E), dtype=np.float32)
    for c in range(N_CORES):
        b, h = divmod(c, 2)
        full[b, h * SQ:(h + 1) * SQ, :] = res.results[c]["out"]
    kernel.last_results = res
    return full
